# revision 1
# baseline (speedup 1.0000x reference)
"""Trainium2 Bass kernel for nn_Decoder (LSTM decoder + Luong attention + vocab proj).

Strategy (8 cores, data-parallel over batch, B_local = 4):
  phase 0: on-device prep per core:
    - embedding gather (indirect DMA) + Xw = X @ W1 + b precomputed for all steps
      (stored in DRAM as [512 tok, 2048], loaded in [128, 2048] chunks / 32 steps)
    - keysT = (memory @ Wm)^T per batch
    - fold attention out-proj into the recurrence:
        Wmod = [U_lstm + Wa_h @ W2 ; Wa_c @ W2]   (g-gate cols pre-scaled x2)
      so per step z_t = Xw_t + h @ Wmod_h + ctx @ Wmod_c, and the Wa GEMM
      leaves the loop.
    - Mproj[b] = mem[b] @ Wmod_c, so ctx @ Wmod_c == alpha_b @ Mproj[b] and the
      context vector itself never has to be materialized inside the loop.
    - step-1 correction corr = h_0 @ (Wa_h @ W2) (since attn_0 = 0, not f(h_0)).
  phase 1: 128 sequential steps; gates via tanh-only ACT table
    (sigma(x) = 0.5*tanh(x/2) + 0.5); attention score via all-pairs matmul with
    an additive block-diagonal mask folded in as an extra matmul; softmax
    weights stored transposed (eTall) for the next step's z and for phase 2.
  phase 2: ctx materialized in batch from eTall, attn = [H|CTX] @ Wa as one
    GEMM, then logits = attn @ Wfc + bfc streaming Wfc from HBM; out rows are
    tokens in (t, b) order.

Big matmuls run in float32r (full-rate fp32 path on the PE). The BIR verifier
requires fp32r operands to be *produced* rounded: loads use gpsimd cast-DMAs
(f32 -> f32r) and compute producers write f32r-typed tiles.
"""

import sys

for _p in ("/opt/trn_rl_repo",):
    if _p not in sys.path:
        sys.path.insert(0, _p)

import numpy as np

B, T, V, D, U = 32, 128, 32000, 256, 512
VO = V + 1
NCORES = 8
BL = B // NCORES  # 4
G = 4 * U  # 2048
NTOK = BL * T  # 512 tokens per core
HT_W = 4 * (T + 1)  # 516 columns per u-chunk in hT buffer

_cache = {}


def _build(n_steps=T, debug=False):
    import concourse.bacc as bacc
    import concourse.bass as bass
    import concourse.mybir as mybir
    import concourse.tile as tile
    from concourse.masks import make_identity

    f32 = mybir.dt.float32
    f32r = mybir.dt.float32  # fp32-grade needed: the attention recurrence amplifies fp32r noise ~e^0.05t
    i32 = mybir.dt.int32
    AX = mybir.AxisListType
    OP = mybir.AluOpType
    AF = mybir.ActivationFunctionType

    try:
        import concourse.tile_utils as _tu

        if getattr(_tu, "max_sbuf_usage", 0) < 204 * 1024:
            _tu.max_sbuf_usage = 204 * 1024
    except Exception:
        pass

    nc = bacc.Bacc(None, target_bir_lowering=False)

    tok_ids = nc.dram_tensor("tok_ids", [NTOK, 1], i32, kind="ExternalInput")
    mem_d = nc.dram_tensor("mem", [BL, T, U], f32, kind="ExternalInput")
    enc_ht_d = nc.dram_tensor("enc_ht", [U, BL], f32, kind="ExternalInput")
    enc_c_d = nc.dram_tensor("enc_c", [BL, U], f32, kind="ExternalInput")
    E_d = nc.dram_tensor("E", [V, D], f32, kind="ExternalInput")
    Wm_d = nc.dram_tensor("Wm", [U, U], f32, kind="ExternalInput")
    W1_d = nc.dram_tensor("W1", [D, G], f32, kind="ExternalInput")
    W2_d = nc.dram_tensor("W2", [U, G], f32, kind="ExternalInput")
    Ul_d = nc.dram_tensor("Ul", [U, G], f32, kind="ExternalInput")
    bl_d = nc.dram_tensor("bl", [1, G], f32, kind="ExternalInput")
    Wa_d = nc.dram_tensor("Wa", [2 * U, U], f32, kind="ExternalInput")
    Wfc_d = nc.dram_tensor("Wfc", [U, VO], f32, kind="ExternalInput")
    bfc_d = nc.dram_tensor("bfc", [1, VO], f32, kind="ExternalInput")
    out_d = nc.dram_tensor("out", [NTOK, VO], f32, kind="ExternalOutput")
    dbg_d = None
    if debug:
        dbg_d = {
            nm: nc.dram_tensor(f"dbg_{nm}", shp, f32, kind="ExternalOutput")
            for nm, shp in [
                ("htall", [128, 4, 4 * (n_steps + 1)]),
                ("eTall", [128, 16 * n_steps]),
                ("wmod0", [128, G]),
                ("kT0", [128, BL * T]),
                ("mpack", [128, BL * U]),
                ("corr", [BL, G]),
                ("mproj0", [128, G]),
                ("mneg", [BL, BL * T]),
                ("xw0", [128, G]),
            ]
        }

    n_chunks = (n_steps * BL + 127) // 128

    with tile.TileContext(nc) as tc:
        # ------------------------------------------------------------------
        # persistent pool
        # ------------------------------------------------------------------
        per_cm = tc.tile_pool(name="per", bufs=1)
        per = per_cm.__enter__()
        dram_cm = tc.tile_pool(name="dram", bufs=1, space="DRAM")
        dram = dram_cm.__enter__()

        wmod = [per.tile([128, G], f32r, tag=f"wmod{k}", name=f"wmod{k}") for k in range(4)]
        kT = [per.tile([128, BL * T], f32r, tag=f"kT{j}", name=f"kT{j}") for j in range(4)]
        mpack = per.tile([128, BL * U], f32r, tag="mpack")  # [t, (b,u)]
        htall = per.tile([128, 4 * HT_W], f32r, tag="htall")
        eTall = per.tile([128, 16 * T], f32r, tag="eTall")
        corr = per.tile([BL, G], f32r, tag="corr")
        I4 = per.tile([4, 4], f32, tag="I4")        # f32: transpose identity
        I4r = per.tile([4, 4], f32r, tag="I4r")     # f32r: matmul lhsT
        I4nr = per.tile([4, 4], f32r, tag="I4nr")
        I128 = per.tile([128, 128], f32, tag="I128")
        I128r = per.tile([128, 128], f32r, tag="I128r")
        ones1 = per.tile([1, 128], f32r, tag="ones1")
        mneg = per.tile([BL, BL * T], f32r, tag="mneg")
        mnegf = per.tile([BL, BL * T], f32, tag="mnegf")

        make_identity(nc, I4[:])
        make_identity(nc, I128[:])
        nc.vector.tensor_copy(I4r[:], I4[:])
        nc.vector.tensor_scalar_mul(I4nr[:], I4[:], -1.0)
        nc.vector.tensor_copy(I128r[:], I128[:])
        onesf = per.tile([1, 128], f32, tag="onesf")
        nc.gpsimd.memset(onesf[:], 1.0)
        nc.vector.tensor_copy(ones1[:], onesf[:])
        # block-diagonal additive mask: 0 on own 128-block, -1e30 elsewhere.
        # iota m = y - T*x, then mask = (1[m>=0]*1[m<=T-1] - 1) * 1e30.
        miot = per.tile([BL, BL * T], f32, tag="miot")
        nc.gpsimd.iota(
            miot[:], pattern=[[1, BL * T]], base=0, channel_multiplier=-T,
            allow_small_or_imprecise_dtypes=True,
        )
        ma = per.tile([BL, BL * T], f32, tag="ma")
        nc.vector.tensor_scalar(ma[:], miot[:], 0.0, None, op0=OP.is_ge)
        nc.vector.tensor_scalar(mnegf[:], miot[:], float(T - 1), None, op0=OP.is_le)
        nc.vector.tensor_tensor(ma[:], ma[:], mnegf[:], op=OP.mult)
        nc.vector.tensor_scalar(mneg[:], ma[:], -1.0, 1e30, op0=OP.add, op1=OP.mult)

        # hT layout: htall[:, HT_W*j + 4*t + b] = h_t[b, 128*j + u']
        def hT_cols(j, t0, ncols):
            v = htall[:].rearrange("p (j s) -> p j s", j=4)
            return v[:, j, 4 * t0 : 4 * t0 + ncols]

        # init h_0 = enc_h (transposed on host): enc_ht [U, BL]
        src_h0 = enc_ht_d[:].rearrange("(j p) b -> p j b", j=4)
        dst_h0 = htall[:].rearrange("p (j s) -> p j s", j=4)[:, :, 0:BL]
        nc.gpsimd.dma_start(dst_h0, src_h0)

        # memory pack: mpack[:, U*b + u] = mem[b, t, u]
        for b in range(BL):
            nc.gpsimd.dma_start(mpack[:, U * b : U * (b + 1)], mem_d[b])

        xw_dram = dram.tile([NTOK, G], f32, name="xw_dram")

        # ------------------------------------------------------------------
        # phase 0a: embedding gather + Xw = X @ W1 + bl (g cols x2); memT;
        # keysT = (mem @ Wm)^T
        # ------------------------------------------------------------------
        mproj_cm = tc.tile_pool(name="mprojp", bufs=1)
        mprojp = mproj_cm.__enter__()
        mproj = [mprojp.tile([128, G], f32r, tag=f"mproj{b}", name=f"mproj{b}") for b in range(BL)]
        mtv_cm = tc.tile_pool(name="mtvp", bufs=1)
        mtvp = mtv_cm.__enter__()
        mtv = [mtvp.tile([128, BL * 128], f32r, tag=f"mtv{v}", name=f"mtv{v}") for v in range(4)]
        wmodc_cm = tc.tile_pool(name="wmodcp", bufs=1)
        wmodcp = wmodc_cm.__enter__()
        wmod += [wmodcp.tile([128, G], f32r, tag=f"wmod{k}", name=f"wmod{k}") for k in range(4, 8)]

        with (
            tc.tile_pool(name="p0a", bufs=2) as p0a,
            tc.tile_pool(name="p0a1", bufs=1) as p0a1,
            tc.tile_pool(name="ps0", bufs=2, space="PSUM") as ps0,
        ):
            bls = p0a1.tile([1, G], f32r, tag="bls")
            nc.gpsimd.dma_start(bls[:], bl_d[:])
            xt = [p0a1.tile([128, NTOK], f32r, tag=f"xt{k}", name=f"xt{k}") for k in range(2)]

            for c in range(NTOK // 128):
                ids_c = p0a.tile([128, 1], i32, tag="ids")
                nc.sync.dma_start(ids_c[:], tok_ids[128 * c : 128 * (c + 1)])
                x_c = p0a.tile([128, D], f32, tag="xc")
                nc.gpsimd.indirect_dma_start(
                    out=x_c[:],
                    out_offset=None,
                    in_=E_d[:],
                    in_offset=bass.IndirectOffsetOnAxis(ap=ids_c[:, :1], axis=0),
                )
                for k in range(2):
                    pt = ps0.tile([128, 128], f32, tag="pt0")
                    nc.tensor.transpose(pt[:], x_c[:, 128 * k : 128 * (k + 1)], I128[:])
                    nc.vector.tensor_copy(xt[k][:, 128 * c : 128 * (c + 1)], pt[:])

            for q in range(4):
                w1q = [
                    p0a.tile([128, 512], f32r, tag="w1q", name=f"w1q{q}_{k}")
                    for k in range(2)
                ]
                for k in range(2):
                    nc.gpsimd.dma_start(
                        w1q[k][:],
                        W1_d[128 * k : 128 * (k + 1), 512 * q : 512 * (q + 1)],
                    )
                for c in range(NTOK // 128):
                    pz0 = ps0.tile([128, 512], f32, tag="pz0")
                    for k in range(2):
                        nc.tensor.matmul(
                            pz0[:],
                            xt[k][:, 128 * c : 128 * (c + 1)],
                            w1q[k][:],
                            start=(k == 0),
                            stop=False,
                        )
                    nc.tensor.matmul(
                        pz0[:],
                        ones1[:1, :128],
                        bls[:1, 512 * q : 512 * (q + 1)],
                        start=False,
                        stop=True,
                    )
                    st = p0a.tile([128, 512], f32, tag="xwst")
                    nc.scalar.activation(
                        st[:], pz0[:], AF.Copy, bias=0.0, scale=2.0 if q == 2 else 1.0
                    )
                    nc.sync.dma_start(
                        xw_dram[128 * c : 128 * (c + 1), 512 * q : 512 * (q + 1)], st[:]
                    )


            # memT: mtv[vc][:, 128*b + t] = mem[b, t, 128*vc + v']
            for b in range(BL):
                memf = p0a.tile([128, U], f32, tag="memf", name=f"memf{b}")
                nc.sync.dma_start(memf[:], mem_d[b])
                for vc in range(4):
                    pt = ps0.tile([128, 128], f32, tag="pt0")
                    nc.tensor.transpose(
                        pt[:], memf[:, 128 * vc : 128 * (vc + 1)], I128[:]
                    )
                    nc.vector.tensor_copy(mtv[vc][:, 128 * b : 128 * (b + 1)], pt[:])

            # keysT
            wms = [p0a1.tile([128, U], f32r, tag=f"wms{k}", name=f"wms{k}") for k in range(4)]
            for k in range(4):
                nc.gpsimd.dma_start(wms[k][:], Wm_d[128 * k : 128 * (k + 1)])
            for j in range(4):
                for b in range(BL):
                    pk = ps0.tile([128, 128], f32, tag="pt0")
                    for vt in range(4):
                        nc.tensor.matmul(
                            pk[:],
                            wms[vt][:, 128 * j : 128 * (j + 1)],
                            mtv[vt][:, 128 * b : 128 * (b + 1)],
                            start=(vt == 0),
                            stop=(vt == 3),
                        )
                    nc.vector.tensor_copy(kT[j][:, 128 * b : 128 * (b + 1)], pk[:])

        # ------------------------------------------------------------------
        # phase 0c: Wmod = [Ul + Wa_h @ W2 ; Wa_c @ W2], g cols x2; corr
        # ------------------------------------------------------------------
        with (
            tc.tile_pool(name="p0c", bufs=1) as p0c,
            tc.tile_pool(name="p0cr", bufs=2) as p0cr,
            tc.tile_pool(name="p0w2", bufs=4) as p0w2,
            tc.tile_pool(name="ps0c", bufs=2, space="PSUM") as ps0c,
        ):
            was = [p0c.tile([128, U], f32r, tag=f"was{k}", name=f"was{k}") for k in range(8)]
            for k in range(8):
                nc.gpsimd.dma_start(was[k][:], Wa_d[128 * k : 128 * (k + 1)])
            wat = [p0c.tile([128, 2 * U], f32r, tag=f"wat{q}", name=f"wat{q}") for q in range(4)]
            for k in range(8):
                for q in range(4):
                    pt = ps0c.tile([128, 128], f32, tag="ptc")
                    # transpose reads the f32r tile as f32 (same bits)
                    nc.tensor.transpose(
                        pt[:], was[k][:, 128 * q : 128 * (q + 1)].bitcast(f32), I128[:]
                    )
                    nc.vector.tensor_copy(wat[q][:, 128 * k : 128 * (k + 1)], pt[:])

            # enc_ht as lhsT tiles: ehts[:, 4*kt + b]
            ehts = p0c.tile([128, 16], f32r, tag="ehts")
            nc.gpsimd.dma_start(
                ehts[:].rearrange("p (k b) -> p k b", k=4),
                enc_ht_d[:].rearrange("(k p) b -> p k b", k=4),
            )

            # corr: s = h0 @ Wa_h ; corr = s @ W2 (g cols x2)
            ps_s = ps0c.tile([4, 512], f32, tag="ps_s")
            for kt in range(4):
                nc.tensor.matmul(
                    ps_s[:],
                    ehts[:, 4 * kt : 4 * kt + 4],
                    was[kt][:],
                    start=(kt == 0),
                    stop=(kt == 3),
                )
            s_sb = p0c.tile([4, 512], f32, tag="s_sb")
            nc.vector.tensor_copy(s_sb[:], ps_s[:])
            stT = p0c.tile([128, 16], f32r, tag="stT")
            for j in range(4):
                pt = ps0c.tile([128, 16], f32, tag="pts")
                nc.tensor.transpose(
                    pt[:, 4 * j : 4 * j + 4], s_sb[:, 128 * j : 128 * (j + 1)], I4[:]
                )
                nc.vector.tensor_copy(stT[:, 4 * j : 4 * j + 4], pt[:, 4 * j : 4 * j + 4])

            # Mfold rows chunk mc (q-outer so W2 slices are loaded once)
            for q in range(4):
                w2q = [
                    p0w2.tile([128, 512], f32r, tag="w2q", name=f"w2q{q}_{kt}")
                    for kt in range(4)
                ]
                for kt in range(4):
                    nc.gpsimd.dma_start(
                        w2q[kt][:],
                        W2_d[128 * kt : 128 * (kt + 1), 512 * q : 512 * (q + 1)],
                    )
                for mc in range(8):
                    pm = ps0c.tile([128, 512], f32, tag="pm")
                    for kt in range(4):
                        nc.tensor.matmul(
                            pm[:],
                            wat[kt][:, 128 * mc : 128 * (mc + 1)],
                            w2q[kt][:],
                            start=(kt == 0),
                            stop=(kt == 3),
                        )
                    dst = wmod[mc][:, 512 * q : 512 * (q + 1)]
                    scl = 2.0 if q == 2 else 1.0
                    if mc < 4:
                        # h rows: Ul chunk + Mfold (then g-scale)
                        ul_t = p0cr.tile([128, 512], f32, tag="ul")
                        nc.sync.dma_start(
                            ul_t[:],
                            Ul_d[128 * mc : 128 * (mc + 1), 512 * q : 512 * (q + 1)],
                        )
                        if q == 2:
                            tmp = p0cr.tile([128, 512], f32, tag="gtmp")
                            nc.vector.tensor_tensor(tmp[:], pm[:], ul_t[:], op=OP.add)
                            nc.vector.tensor_scalar_mul(dst, tmp[:], 2.0)
                        else:
                            nc.vector.tensor_tensor(dst, pm[:], ul_t[:], op=OP.add)
                    else:
                        nc.scalar.activation(dst, pm[:], AF.Copy, bias=0.0, scale=scl)

                # corr chunk q while w2q is resident
                pc = ps0c.tile([4, 512], f32, tag="ps_s")
                for kt in range(4):
                    nc.tensor.matmul(
                        pc[:],
                        stT[:, 4 * kt : 4 * kt + 4],
                        w2q[kt][:],
                        start=(kt == 0),
                        stop=(kt == 3),
                    )
                nc.scalar.activation(
                    corr[:, 512 * q : 512 * (q + 1)],
                    pc[:],
                    AF.Copy,
                    bias=0.0,
                    scale=2.0 if q == 2 else 1.0,
                )

        # ------------------------------------------------------------------
        # phase 0d: Mproj[b] = mem[b] @ Wmod_c  (uses mtv, then frees it)
        # ------------------------------------------------------------------
        with tc.tile_pool(name="ps0d", bufs=2, space="PSUM") as ps0d:
            for b in range(BL):
                for q in range(4):
                    pm = ps0d.tile([128, 512], f32, tag="pmd")
                    for kt in range(4):
                        nc.tensor.matmul(
                            pm[:],
                            mtv[kt][:, 128 * b : 128 * (b + 1)],
                            wmod[4 + kt][:, 512 * q : 512 * (q + 1)],
                            start=(kt == 0),
                            stop=(kt == 3),
                        )
                    nc.vector.tensor_copy(mproj[b][:, 512 * q : 512 * (q + 1)], pm[:])
        wmodc_cm.__exit__(None, None, None)
        mtv_cm.__exit__(None, None, None)

        # ------------------------------------------------------------------
        # phase 1: the recurrence
        # ------------------------------------------------------------------
        with (
            tc.tile_pool(name="wk", bufs=1) as wk,
            tc.tile_pool(name="xwp", bufs=2) as xwp,
            tc.tile_pool(name="cst", bufs=2) as cst,
            tc.tile_pool(name="pz", bufs=1, space="PSUM") as pzp,
            tc.tile_pool(name="pat", bufs=1, space="PSUM") as patp,
            tc.tile_pool(name="ptr", bufs=3, space="PSUM") as ptrp,
        ):
            c_prev = cst.tile([BL, U], f32, tag="c")
            nc.sync.dma_start(c_prev[:], enc_c_d[:])

            xwc = {}

            def load_xw_chunk(c):
                tl = xwp.tile([128, G], f32r, tag="xwc", name=f"xwc{c}")
                rows = min(128, NTOK - 128 * c)
                nc.gpsimd.dma_start(tl[:rows, :], xw_dram[128 * c : 128 * c + rows])
                xwc[c] = tl

            load_xw_chunk(0)

            def z_head(t, pz):
                """Xw inject (+ step-1 corr) + h-part of z_t: everything that
                only needs hT_{t-1}; emitted early so PE fills stall windows."""
                ch = (t - 1) // 32
                row = 4 * ((t - 1) % 32)
                for q in range(4):
                    zq = pz[:, 512 * q : 512 * (q + 1)]
                    nc.tensor.matmul(
                        zq,
                        I128r[:, row : row + 4],
                        xwc[ch][:, 512 * q : 512 * (q + 1)],
                        start=True,
                        stop=False,
                    )
                    if t == 1:
                        nc.tensor.matmul(
                            zq, I4nr[:], corr[:, 512 * q : 512 * (q + 1)],
                            start=False, stop=False,
                        )
                for kt in range(4):
                    for q in range(4):
                        nc.tensor.matmul(
                            pz[:, 512 * q : 512 * (q + 1)],
                            hT_cols(kt, t - 1, 4),
                            wmod[kt][:, 512 * q : 512 * (q + 1)],
                            start=False,
                            stop=(t == 1 and kt == 3),
                        )

            def z_tail(t, pz):
                """ctx contribution via alpha_{t-1} @ Mproj[b] (eT block b has
                e_b in col b, zeros elsewhere)."""
                ec = 16 * (t - 2)
                for b in range(BL):
                    for q in range(4):
                        nc.tensor.matmul(
                            pz[:, 512 * q : 512 * (q + 1)],
                            eTall[:, ec + 4 * b : ec + 4 * b + 4],
                            mproj[b][:, 512 * q : 512 * (q + 1)],
                            start=False,
                            stop=(b == 3),
                        )

            pz_cur = pzp.tile([BL, G], f32, tag="pz", name="pz1")
            z_head(1, pz_cur)

            for t in range(1, n_steps + 1):
                if t % 32 == 2 and (t - 1) // 32 + 1 < n_chunks:
                    load_xw_chunk((t - 1) // 32 + 1)

                pz = pz_cur

                # --- gates (tanh table only) ---
                th = wk.tile([BL, G], f32, tag="th")
                nc.scalar.activation(th[:], pz[:], AF.Tanh, bias=0.0, scale=0.5)

                # score-pairs mask contribution: no step deps; runs mid-gates
                # and keeps the PE from going HAM-cold.
                psc = patp.tile([BL, BL * T], f32, tag="pat")
                nc.tensor.matmul(psc[:], I4r[:], mneg[:], start=True, stop=False)
                sig_i = wk.tile([BL, U], f32, tag="sig_i")
                sig_f = wk.tile([BL, U], f32, tag="sig_f")
                sig_o = wk.tile([BL, U], f32, tag="sig_o")
                nc.vector.tensor_scalar(
                    sig_f[:], th[:, 512:1024], 0.5, 0.5, op0=OP.mult, op1=OP.add
                )
                nc.vector.tensor_scalar(
                    sig_i[:], th[:, 0:512], 0.5, 0.5, op0=OP.mult, op1=OP.add
                )
                nc.vector.tensor_scalar(
                    sig_o[:], th[:, 1536:2048], 0.5, 0.5, op0=OP.mult, op1=OP.add
                )
                m1 = wk.tile([BL, U], f32, tag="m1")
                m2 = wk.tile([BL, U], f32, tag="m2")
                nc.vector.tensor_tensor(m1[:], sig_f[:], c_prev[:], op=OP.mult)
                nc.vector.tensor_tensor(m2[:], sig_i[:], th[:, 1024:1536], op=OP.mult)
                c_new = cst.tile([BL, U], f32, tag="c")
                nc.vector.tensor_tensor(c_new[:], m1[:], m2[:], op=OP.add)
                tc_ = wk.tile([BL, U], f32, tag="tc")
                nc.scalar.activation(tc_[:], c_new[:], AF.Tanh, bias=0.0, scale=1.0)
                h = wk.tile([BL, U], f32, tag="h")
                nc.vector.tensor_tensor(h[:], sig_o[:], tc_[:], op=OP.mult)
                c_prev = c_new

                # --- hT via PE transposes -> htall cols 4t ---
                pht = ptrp.tile([128, 16], f32, tag="ptr")
                for j in range(4):
                    nc.tensor.transpose(
                        pht[:, 4 * j : 4 * j + 4], h[:, 128 * j : 128 * (j + 1)], I4[:]
                    )
                nc.vector.tensor_copy(
                    htall[:].rearrange("p (j s) -> p j s", j=4)[
                        :, :, 4 * t : 4 * t + 4
                    ],
                    pht[:].rearrange("p (j b) -> p j b", j=4),
                )

                # --- score pairs [b, (b', t')] (mask already in psc) ---
                for kt in range(4):
                    nc.tensor.matmul(
                        psc[:],
                        hT_cols(kt, t, 4),
                        kT[kt][:],
                        start=False,
                        stop=(kt == 3),
                    )

                # --- z_{t+1} head: fills the PE while softmax runs ---
                if t < n_steps:
                    pz_cur = pzp.tile([BL, G], f32, tag="pz", name=f"pz{t + 1}")
                    z_head(t + 1, pz_cur)

                # --- masked softmax straight off PSUM ---
                nmax = wk.tile([BL, 1], f32, tag="nmax")
                nc.vector.tensor_reduce(
                    nmax[:], psc[:], axis=AX.X, op=OP.max, negate=True
                )
                e = wk.tile([BL, BL * T], f32, tag="e")
                ssum = wk.tile([BL, 1], f32, tag="ssum")
                nc.scalar.activation(
                    e[:], psc[:], AF.Exp, bias=nmax[:, :1], scale=1.0,
                    accum_out=ssum[:, :1],
                )
                rec = wk.tile([BL, 1], f32, tag="rec")
                nc.vector.reciprocal(rec[:], ssum[:])
                e2 = wk.tile([BL, BL * T], f32, tag="e2")
                nc.vector.tensor_scalar(
                    e2[:], e[:], rec[:, :1], None, op0=OP.mult
                )

                # --- eT blocks -> eTall ---
                pet = ptrp.tile([128, 16], f32, tag="ptr")
                for q in range(BL):
                    nc.tensor.transpose(
                        pet[:, 4 * q : 4 * q + 4], e2[:, T * q : T * (q + 1)], I4[:]
                    )
                nc.vector.tensor_copy(eTall[:, 16 * (t - 1) : 16 * t], pet[:])

                # --- z_{t+1} tail: ctx contribution, needs eTall of step t ---
                if t < n_steps:
                    z_tail(t + 1, pz_cur)

            if debug:
                ht_w = htall[:].rearrange("p (j s) -> p j s", j=4)[
                    :, :, 0 : 4 * (n_steps + 1)
                ]
                nc.gpsimd.dma_start(dbg_d["htall"][:], ht_w.bitcast(f32))
                nc.gpsimd.dma_start(
                    dbg_d["eTall"][:], eTall[:, 0 : 16 * n_steps].bitcast(f32)
                )
                for nm, tl in [
                    ("wmod0", wmod[0]), ("kT0", kT[0]), ("mpack", mpack),
                    ("corr", corr), ("mproj0", mproj[0]), ("mneg", mneg),
                    ("xw0", xwc[0]),
                ]:
                    nc.gpsimd.dma_start(dbg_d[nm][:], tl[:].bitcast(f32))

        mproj_cm.__exit__(None, None, None)

        # ------------------------------------------------------------------
        # phase 2: ctxT from eTall; attn = [H|CTX] @ Wa; logits = attn @ Wfc
        # ------------------------------------------------------------------
        with (
            tc.tile_pool(name="p2", bufs=1) as p2,
            tc.tile_pool(name="p2r", bufs=3) as p2r,
            tc.tile_pool(name="ps2", bufs=4, space="PSUM") as ps2,
        ):
            ntok = BL * n_steps

            # ctxT: ct2[j][:, 4*k + b] = ctx_{k+1}[b, 128j + u']
            ct2 = [p2.tile([128, NTOK], f32r, tag=f"ct2{j}", name=f"ct2{j}") for j in range(4)]
            eview = eTall[:].rearrange("p (t s) -> p t s", s=16)
            for j in range(4):
                for b in range(BL):
                    pc2 = ps2.tile([128, T], f32, tag="pc2", bufs=2)
                    nc.tensor.matmul(
                        pc2[:, :n_steps],
                        mpack[:, U * b + 128 * j : U * b + 128 * (j + 1)],
                        eview[:, 0:n_steps, 4 * b + b],
                        start=True,
                        stop=True,
                    )
                    dst = ct2[j][:].rearrange("p (k b) -> p k b", b=4)[
                        :, 0:n_steps, b
                    ]
                    nc.vector.tensor_copy(dst, pc2[:, :n_steps])

            wax = [p2.tile([128, U], f32r, tag=f"wax{k}", name=f"wax{k}") for k in range(8)]
            for k in range(8):
                nc.gpsimd.dma_start(wax[k][:], Wa_d[128 * k : 128 * (k + 1)])
            att = [p2.tile([128, NTOK], f32r, tag=f"att{j}", name=f"att{j}") for j in range(4)]
            for j in range(4):
                pa = ps2.tile([128, 512], f32, tag="pa", bufs=2)
                for kt in range(8):
                    if kt < 4:
                        src = htall[:].rearrange("p (jj s) -> p jj s", jj=4)[
                            :, kt, 4 : 4 + ntok
                        ]
                    else:
                        src = ct2[kt - 4][:, :ntok]
                    nc.tensor.matmul(
                        pa[:, :ntok],
                        wax[kt][:, 128 * j : 128 * (j + 1)],
                        src,
                        start=(kt == 0),
                        stop=(kt == 7),
                    )
                nc.vector.tensor_copy(att[j][:, :ntok], pa[:, :ntok])

            NCH = (VO + 511) // 512  # 63
            for nci in range(NCH):
                # last chunk overlaps the previous one so every chunk is a
                # full 512 wide (fp32r matmul needs aligned free dims)
                n0 = min(512 * nci, VO - 512)
                ncols = 512
                wf = p2r.tile([128, 4, 512], f32r, tag="wf")
                nc.gpsimd.dma_start(
                    wf[:, :, :ncols],
                    Wfc_d[:, n0 : n0 + ncols].rearrange("(k p) n -> p k n", k=4),
                )
                bfc_t = p2r.tile([1, 512], f32r, tag="bfc")
                nc.gpsimd.dma_start(bfc_t[:1, :ncols], bfc_d[:1, n0 : n0 + ncols])
                for mt in range((ntok + 127) // 128):
                    mrows = min(128, ntok - 128 * mt)
                    pl = ps2.tile([128, 512], f32, tag="pl", bufs=4)
                    for kt in range(4):
                        nc.tensor.matmul(
                            pl[:mrows, :ncols],
                            att[kt][:, 128 * mt : 128 * mt + mrows],
                            wf[:, kt, :ncols],
                            start=(kt == 0),
                            stop=False,
                        )
                    nc.tensor.matmul(
                        pl[:mrows, :ncols],
                        ones1[:1, :mrows],
                        bfc_t[:1, :ncols],
                        start=False,
                        stop=True,
                    )
                    ot = p2r.tile([128, 512], f32, tag="ot")
                    nc.vector.tensor_copy(ot[:mrows, :ncols], pl[:mrows, :ncols])
                    nc.sync.dma_start(
                        out_d[128 * mt : 128 * mt + mrows, n0 : n0 + ncols],
                        ot[:mrows, :ncols],
                    )

        dram_cm.__exit__(None, None, None)
        per_cm.__exit__(None, None, None)

    nc.compile()
    return nc


def _shard_inputs(inputs, memory, enc_h, enc_c, E, Wm, W_lstm, U_lstm, b_lstm, Wa, Wfc, bfc):
    inputs = np.ascontiguousarray(inputs)
    shared = {
        "E": np.ascontiguousarray(E, np.float32),
        "Wm": np.ascontiguousarray(Wm, np.float32),
        "W1": np.ascontiguousarray(W_lstm[:D], np.float32),
        "W2": np.ascontiguousarray(W_lstm[D:], np.float32),
        "Ul": np.ascontiguousarray(U_lstm, np.float32),
        "bl": np.ascontiguousarray(b_lstm.reshape(1, G), np.float32),
        "Wa": np.ascontiguousarray(Wa, np.float32),
        "Wfc": np.ascontiguousarray(Wfc, np.float32),
        "bfc": np.ascontiguousarray(bfc.reshape(1, VO), np.float32),
    }
    in_maps = []
    for rk in range(NCORES):
        sl = slice(BL * rk, BL * (rk + 1))
        m = dict(shared)
        m["tok_ids"] = np.ascontiguousarray(
            inputs[sl].T.reshape(NTOK, 1), np.int32
        )
        m["mem"] = np.ascontiguousarray(memory[sl], np.float32)
        m["enc_ht"] = np.ascontiguousarray(enc_h[sl].T, np.float32)
        m["enc_c"] = np.ascontiguousarray(enc_c[sl], np.float32)
        in_maps.append(m)
    return in_maps


def kernel(**inputs):
    from concourse.bass_utils import run_bass_kernel_spmd

    if "nc" not in _cache:
        _cache["nc"] = _build(T)
    nc = _cache["nc"]

    in_maps = _shard_inputs(**inputs)
    res = run_bass_kernel_spmd(nc, in_maps, core_ids=list(range(NCORES)))
    outs = []
    for rk in range(NCORES):
        o = res.results[rk]["out"]  # [512, 32001], rows (t, b)
        outs.append(o.reshape(T, BL, VO).transpose(1, 0, 2))
    full = np.concatenate(outs, axis=0)  # [32, 128, 32001]
    return full.astype(np.float32)



# revision 22
# speedup vs baseline: 1.4702x; 1.4702x over previous
"""Trainium2 Bass kernel for nn_Decoder (LSTM decoder + Luong attention + vocab proj).

Strategy (8 cores, data-parallel over batch, B_local = 4):
  All recurrence-feeding matmuls run as 3-term bf16 hi/lo splits
  (a@b ~= ah@bh + ah@bl + al@bh, each 1 cyc/row on the PE vs 4 for fp32;
  dropped al@bl term ~2^-16 relative, well inside the 2e-2 tolerance as
  the chaotic recurrence amplifies per-step noise ~1e4x).

  phase 0: on-device prep per core (bf16-split GEMMs):
    - embedding gather (indirect DMA on host-split E_hi/E_lo) +
      Xw = X @ W1 + bl for all steps, kept in SBUF as bf16 hi/lo chunks
    - kT = 0.5*(mem @ Wm)^T per batch (0.5 folds the doubled-h state)
    - Wmod = [0.5*(Ul + Wa_h @ W2) ; Wa_c @ W2] (g cols x2 pre-scaling),
      so per step z_t = Xw_t + H @ Wmod_h + alpha @ Mproj, H = 2h
    - Mproj[b] = mem[b] @ Wmod_c
    - corr = h0 @ (Wa_h @ W2) step-1 correction (attn_0 = 0)
  phase 1: 128 sequential steps with doubled states C=2c, H=2h:
    gates need only tanh tables: u=(th_f+1)*C, v=(th_i+1)*th_g,
    C'=0.5u+v, H=(th_o+1)*tanh(0.5C') - 4 fused DVE ops + 2 ACT.
    score via all-pairs matmul with additive block-diag mask; softmax
    weights transposed into eTall (bf16 hi/lo) for the next z and phase 2.
  phase 2 (single bf16, feeds only the final logits): ctx from eTall_hi,
    attn = [H|CTX] @ Wa_att (h rows pre-halved), logits = attn @ Wfc(bf16)
    streamed from HBM, output written bf16 (host upcasts + adds bfc).
"""

import sys

for _p in ("/opt/trn_rl_repo",):
    if _p not in sys.path:
        sys.path.insert(0, _p)

import numpy as np

B, T, V, D, U = 32, 128, 32000, 256, 512
VO = V + 1
NCORES = 8
BL = B // NCORES  # 4
G = 4 * U  # 2048
NTOK = BL * T  # 512 tokens per core
HT_W = 4 * (T + 1)  # 516 columns per u-chunk in hT buffer

_cache = {}


def _build(n_steps=T):
    import concourse.bacc as bacc
    import concourse.bass as bass
    import concourse.mybir as mybir
    import concourse.tile as tile
    from concourse.masks import make_identity

    f32 = mybir.dt.float32
    bf16 = mybir.dt.bfloat16
    i32 = mybir.dt.int32
    AX = mybir.AxisListType
    OP = mybir.AluOpType
    AF = mybir.ActivationFunctionType

    try:
        import concourse.tile_utils as _tu

        if getattr(_tu, "max_sbuf_usage", 0) < 204 * 1024:
            _tu.max_sbuf_usage = 204 * 1024
    except Exception:
        pass

    nc = bacc.Bacc(None, target_bir_lowering=False)

    tok_ids = nc.dram_tensor("tok_ids", [NTOK, 1], i32, kind="ExternalInput")
    mem_d = nc.dram_tensor("mem", [BL, T, U], f32, kind="ExternalInput")
    # 2*enc_h transposed, bf16 hi/lo
    enc_hth_d = nc.dram_tensor("enc_hth", [U, BL], bf16, kind="ExternalInput")
    enc_htl_d = nc.dram_tensor("enc_htl", [U, BL], bf16, kind="ExternalInput")
    enc_c2_d = nc.dram_tensor("enc_c2", [BL, U], f32, kind="ExternalInput")
    Eh_d = nc.dram_tensor("Eh", [V, D], bf16, kind="ExternalInput")
    El_d = nc.dram_tensor("El", [V, D], bf16, kind="ExternalInput")
    Wmh_d = nc.dram_tensor("Wmh", [U, U], bf16, kind="ExternalInput")
    Wml_d = nc.dram_tensor("Wml", [U, U], bf16, kind="ExternalInput")
    W1h_d = nc.dram_tensor("W1h", [D, G], bf16, kind="ExternalInput")
    W1l_d = nc.dram_tensor("W1l", [D, G], bf16, kind="ExternalInput")
    W2h_d = nc.dram_tensor("W2h", [U, G], bf16, kind="ExternalInput")
    W2l_d = nc.dram_tensor("W2l", [U, G], bf16, kind="ExternalInput")
    Ul_d = nc.dram_tensor("Ul", [U, G], f32, kind="ExternalInput")
    blh_d = nc.dram_tensor("blh", [1, G], bf16, kind="ExternalInput")
    bll_d = nc.dram_tensor("bll", [1, G], bf16, kind="ExternalInput")
    Wah_d = nc.dram_tensor("Wah", [2 * U, U], bf16, kind="ExternalInput")
    Wal_d = nc.dram_tensor("Wal", [2 * U, U], bf16, kind="ExternalInput")
    Waatt_d = nc.dram_tensor("Waatt", [2 * U, U], bf16, kind="ExternalInput")
    Wfc_d = nc.dram_tensor("Wfc", [U, VO], bf16, kind="ExternalInput")
    out_d = nc.dram_tensor("out", [NTOK, VO], bf16, kind="ExternalOutput")

    with tile.TileContext(nc) as tc:
        # ------------------------------------------------------------------
        # persistent pool
        # ------------------------------------------------------------------
        per_cm = tc.tile_pool(name="per", bufs=1)
        per = per_cm.__enter__()

        wmodh = [per.tile([128, G], bf16, tag=f"wmodh{k}", name=f"wmodh{k}") for k in range(4)]
        wmodl = [per.tile([128, G], bf16, tag=f"wmodl{k}", name=f"wmodl{k}") for k in range(4)]
        kTh = [per.tile([128, BL * T], bf16, tag=f"kTh{j}", name=f"kTh{j}") for j in range(4)]
        kTl = [per.tile([128, BL * T], bf16, tag=f"kTl{j}", name=f"kTl{j}") for j in range(4)]
        mpack = per.tile([128, BL * U], bf16, tag="mpack")  # [t, (b,u)] single bf16
        htallh = per.tile([128, 4 * HT_W], bf16, tag="htallh")
        htalll = per.tile([128, 4 * HT_W], bf16, tag="htalll")
        eTallh = per.tile([128, 16 * T], bf16, tag="eTallh")
        eTalll = per.tile([128, 16 * T], bf16, tag="eTalll")
        corrh = per.tile([BL, G], bf16, tag="corrh")
        corrl = per.tile([BL, G], bf16, tag="corrl")
        dram_cm = tc.tile_pool(name="dram", bufs=1, space="DRAM")
        dram = dram_cm.__enter__()
        xwh_dram = dram.tile([NTOK, G], bf16, name="xwh_dram")
        xwl_dram = dram.tile([NTOK, G], bf16, name="xwl_dram")
        I4 = per.tile([4, 4], f32, tag="I4")        # f32: transpose identity
        I4b = per.tile([4, 4], bf16, tag="I4b")     # bf16 matmul lhsT
        I4nb = per.tile([4, 4], bf16, tag="I4nb")
        I128b = per.tile([128, 128], bf16, tag="I128b")
        onesb = per.tile([1, 128], bf16, tag="onesb")
        mnegb = per.tile([BL, BL * T], bf16, tag="mnegb")

        make_identity(nc, I4[:])
        nc.vector.tensor_copy(I4b[:], I4[:])
        nc.vector.tensor_scalar_mul(I4nb[:], I4[:], -1.0)
        with tc.tile_pool(name="pinit", bufs=1) as pinit:
            onesf = pinit.tile([1, 128], f32, tag="onesf")
            nc.gpsimd.memset(onesf[:], 1.0)
            nc.vector.tensor_copy(onesb[:], onesf[:])
            # block-diagonal additive mask: 0 on own 128-block, -1e30 elsewhere.
            miot = pinit.tile([BL, BL * T], f32, tag="miot")
            nc.gpsimd.iota(
                miot[:], pattern=[[1, BL * T]], base=0, channel_multiplier=-T,
                allow_small_or_imprecise_dtypes=True,
            )
            ma = pinit.tile([BL, BL * T], f32, tag="ma")
            mb = pinit.tile([BL, BL * T], f32, tag="mb")
            nc.vector.tensor_scalar(ma[:], miot[:], 0.0, None, op0=OP.is_ge)
            nc.vector.tensor_scalar(mb[:], miot[:], float(T - 1), None, op0=OP.is_le)
            nc.vector.tensor_tensor(ma[:], ma[:], mb[:], op=OP.mult)
            nc.vector.tensor_scalar(mnegb[:], ma[:], -1.0, 1e30, op0=OP.add, op1=OP.mult)

        # hT layout: htall[:, HT_W*j + 4*t + b] = H_t[b, 128*j + u']
        def hT_cols(tl, j, t0, ncols):
            v = tl[:].rearrange("p (j s) -> p j s", j=4)
            return v[:, j, 4 * t0 : 4 * t0 + ncols]

        # init H_0 = 2*enc_h (transposed+split on host)
        for tl, src in ((htallh, enc_hth_d), (htalll, enc_htl_d)):
            nc.gpsimd.dma_start(
                tl[:].rearrange("p (j s) -> p j s", j=4)[:, :, 0:BL],
                src[:].rearrange("(j p) b -> p j b", j=4),
            )

        # ------------------------------------------------------------------
        # phase 0a: mem transposes (mtv hi/lo + mpack), keys, embedding + Xw
        # ------------------------------------------------------------------
        mproj_cm = tc.tile_pool(name="mprojp", bufs=1)
        mprojp = mproj_cm.__enter__()
        mprojh = [mprojp.tile([128, G], bf16, tag=f"mprojh{b}", name=f"mprojh{b}") for b in range(BL)]
        mprojl = [mprojp.tile([128, G], bf16, tag=f"mprojl{b}", name=f"mprojl{b}") for b in range(BL)]
        mtv_cm = tc.tile_pool(name="mtvp", bufs=1)
        mtvp = mtv_cm.__enter__()
        mtvh = [mtvp.tile([128, BL * 128], bf16, tag=f"mtvh{v}", name=f"mtvh{v}") for v in range(4)]
        mtvl = [mtvp.tile([128, BL * 128], bf16, tag=f"mtvl{v}", name=f"mtvl{v}") for v in range(4)]

        with (
            tc.tile_pool(name="p0a", bufs=2) as p0a,
            tc.tile_pool(name="p0a1", bufs=1) as p0a1,
            tc.tile_pool(name="ps0", bufs=2, space="PSUM") as ps0,
        ):
            I128 = p0a1.tile([128, 128], f32, tag="I128")
            make_identity(nc, I128[:])
            nc.vector.tensor_copy(I128b[:], I128[:])

            # memT + mpack
            for b in range(BL):
                memf = p0a.tile([128, U], f32, tag="memf", name=f"memf{b}")
                nc.sync.dma_start(memf[:], mem_d[b])
                nc.vector.tensor_copy(mpack[:, U * b : U * (b + 1)], memf[:])
                for vc in range(4):
                    pt = ps0.tile([128, 128], f32, tag="pt0")
                    nc.tensor.transpose(
                        pt[:], memf[:, 128 * vc : 128 * (vc + 1)], I128[:]
                    )
                    dh = mtvh[vc][:, 128 * b : 128 * (b + 1)]
                    nc.vector.tensor_copy(dh, pt[:])
                    nc.vector.tensor_tensor(
                        mtvl[vc][:, 128 * b : 128 * (b + 1)], pt[:], dh, op=OP.subtract
                    )

            # kT = 0.5 * keysT (0.5 folds H=2h)
            wmsh = [p0a1.tile([128, U], bf16, tag=f"wmsh{k}", name=f"wmsh{k}") for k in range(4)]
            wmsl = [p0a1.tile([128, U], bf16, tag=f"wmsl{k}", name=f"wmsl{k}") for k in range(4)]
            for k in range(4):
                nc.gpsimd.dma_start(wmsh[k][:], Wmh_d[128 * k : 128 * (k + 1)])
                nc.gpsimd.dma_start(wmsl[k][:], Wml_d[128 * k : 128 * (k + 1)])
            for j in range(4):
                for b in range(BL):
                    pk = ps0.tile([128, 128], f32, tag="pt0")
                    nmm = 0
                    for vt in range(4):
                        for lh, rh in (
                            (wmsh[vt], mtvh[vt]),
                            (wmsh[vt], mtvl[vt]),
                            (wmsl[vt], mtvh[vt]),
                        ):
                            nmm += 1
                            nc.tensor.matmul(
                                pk[:],
                                lh[:, 128 * j : 128 * (j + 1)],
                                rh[:, 128 * b : 128 * (b + 1)],
                                start=(nmm == 1),
                                stop=(nmm == 12),
                            )
                    dh = kTh[j][:, 128 * b : 128 * (b + 1)]
                    nc.vector.tensor_scalar_mul(dh, pk[:], 0.5)
                    nc.vector.scalar_tensor_tensor(
                        kTl[j][:, 128 * b : 128 * (b + 1)],
                        pk[:], 0.5, dh, op0=OP.mult, op1=OP.subtract,
                    )

            # embedding gather (hi/lo) + transposes
            xth = [p0a1.tile([128, NTOK], bf16, tag=f"xth{k}", name=f"xth{k}") for k in range(2)]
            xtl = [p0a1.tile([128, NTOK], bf16, tag=f"xtl{k}", name=f"xtl{k}") for k in range(2)]
            for c in range(NTOK // 128):
                ids_c = p0a.tile([128, 1], i32, tag="ids")
                nc.sync.dma_start(ids_c[:], tok_ids[128 * c : 128 * (c + 1)])
                xch = p0a.tile([128, D], bf16, tag="xch")
                xcl = p0a.tile([128, D], bf16, tag="xcl")
                for xc, E_d in ((xch, Eh_d), (xcl, El_d)):
                    nc.gpsimd.indirect_dma_start(
                        out=xc[:],
                        out_offset=None,
                        in_=E_d[:],
                        in_offset=bass.IndirectOffsetOnAxis(ap=ids_c[:, :1], axis=0),
                    )
                for k in range(2):
                    for xc, xt in ((xch, xth), (xcl, xtl)):
                        pt = ps0.tile([128, 128], bf16, tag="ptb")
                        nc.tensor.transpose(
                            pt[:], xc[:, 128 * k : 128 * (k + 1)], I128b[:]
                        )
                        nc.vector.tensor_copy(xt[k][:, 128 * c : 128 * (c + 1)], pt[:])

            # Xw = X @ W1 + bl (g cols x2), kept in SBUF bf16 hi/lo
            blh = p0a1.tile([1, G], bf16, tag="blh")
            bll = p0a1.tile([1, G], bf16, tag="bll")
            nc.gpsimd.dma_start(blh[:], blh_d[:])
            nc.gpsimd.dma_start(bll[:], bll_d[:])
            for q in range(4):
                w1qh = [p0a.tile([128, 512], bf16, tag="w1qh", name=f"w1qh{q}_{k}") for k in range(2)]
                w1ql = [p0a.tile([128, 512], bf16, tag="w1ql", name=f"w1ql{q}_{k}") for k in range(2)]
                for k in range(2):
                    nc.gpsimd.dma_start(
                        w1qh[k][:], W1h_d[128 * k : 128 * (k + 1), 512 * q : 512 * (q + 1)]
                    )
                    nc.gpsimd.dma_start(
                        w1ql[k][:], W1l_d[128 * k : 128 * (k + 1), 512 * q : 512 * (q + 1)]
                    )
                for c in range(NTOK // 128):
                    pz0 = ps0.tile([128, 512], f32, tag="pz0")
                    nc.tensor.matmul(
                        pz0[:], onesb[:1, :128], blh[:1, 512 * q : 512 * (q + 1)],
                        start=True, stop=False,
                    )
                    nc.tensor.matmul(
                        pz0[:], onesb[:1, :128], bll[:1, 512 * q : 512 * (q + 1)],
                        start=False, stop=False,
                    )
                    nmm = 0
                    for k in range(2):
                        for lh, rh in (
                            (xth[k], w1qh[k]),
                            (xth[k], w1ql[k]),
                            (xtl[k], w1qh[k]),
                        ):
                            nmm += 1
                            nc.tensor.matmul(
                                pz0[:],
                                lh[:, 128 * c : 128 * (c + 1)],
                                rh[:],
                                start=False,
                                stop=(nmm == 6),
                            )
                    scl = 2.0 if q == 2 else 1.0
                    sth = p0a.tile([128, 512], bf16, tag="xwsth")
                    stl = p0a.tile([128, 512], bf16, tag="xwstl")
                    nc.vector.tensor_scalar_mul(sth[:], pz0[:], scl)
                    nc.vector.scalar_tensor_tensor(
                        stl[:], pz0[:], scl, sth[:], op0=OP.mult, op1=OP.subtract
                    )
                    nc.sync.dma_start(
                        xwh_dram[128 * c : 128 * (c + 1), 512 * q : 512 * (q + 1)], sth[:]
                    )
                    nc.sync.dma_start(
                        xwl_dram[128 * c : 128 * (c + 1), 512 * q : 512 * (q + 1)], stl[:]
                    )

        # ------------------------------------------------------------------
        # phase 0c: Wmod = [0.5(Ul + Wa_h @ W2) ; Wa_c @ W2], g cols x2; corr
        # ------------------------------------------------------------------
        wmodc_cm = tc.tile_pool(name="wmodcp", bufs=1)
        wmodcp = wmodc_cm.__enter__()
        wmodch = [wmodcp.tile([128, G], bf16, tag=f"wmodch{k}", name=f"wmodch{k}") for k in range(4)]
        wmodcl = [wmodcp.tile([128, G], bf16, tag=f"wmodcl{k}", name=f"wmodcl{k}") for k in range(4)]

        with (
            tc.tile_pool(name="p0c", bufs=1) as p0c,
            tc.tile_pool(name="p0cw", bufs=2) as p0cw,
            tc.tile_pool(name="p0cr", bufs=2) as p0cr,
            tc.tile_pool(name="p0w2", bufs=4) as p0w2,
            tc.tile_pool(name="ps0c", bufs=2, space="PSUM") as ps0c,
            tc.tile_pool(name="ps0s", bufs=1, space="PSUM") as ps0s,
        ):
            # enc_ht as lhsT tiles: ehts[:, 4*kt + b] (holds 2*h0)
            ehtsh = p0c.tile([128, 16], bf16, tag="ehtsh")
            ehtsl = p0c.tile([128, 16], bf16, tag="ehtsl")
            for tl, src in ((ehtsh, enc_hth_d), (ehtsl, enc_htl_d)):
                nc.gpsimd.dma_start(
                    tl[:].rearrange("p (k b) -> p k b", k=4),
                    src[:].rearrange("(k p) b -> p k b", k=4),
                )

            # stream Wa chunks: transposes into wat, s-accumulation for corr
            wath = [p0c.tile([128, 2 * U], bf16, tag=f"wath{q}", name=f"wath{q}") for q in range(4)]
            watl = [p0c.tile([128, 2 * U], bf16, tag=f"watl{q}", name=f"watl{q}") for q in range(4)]
            ps_s = ps0s.tile([4, 512], f32, tag="ps_s")
            for k in range(8):
                wkh = p0cw.tile([128, U], bf16, tag="wkh", name=f"wkh{k}")
                wkl = p0cw.tile([128, U], bf16, tag="wkl", name=f"wkl{k}")
                nc.gpsimd.dma_start(wkh[:], Wah_d[128 * k : 128 * (k + 1)])
                nc.gpsimd.dma_start(wkl[:], Wal_d[128 * k : 128 * (k + 1)])
                for q in range(4):
                    for was, wat in ((wkh, wath), (wkl, watl)):
                        pt = ps0c.tile([128, 128], bf16, tag="ptc")
                        nc.tensor.transpose(
                            pt[:], was[:, 128 * q : 128 * (q + 1)], I128b[:]
                        )
                        # bf16-valued, so the copy is exact
                        nc.vector.tensor_copy(wat[q][:, 128 * k : 128 * (k + 1)], pt[:])
                if k < 4:
                    # corr s-part: s = h0 @ Wa_h (0.5 un-doubles h0)
                    for i_, (lh, rh) in enumerate((
                        (ehtsh, wkh),
                        (ehtsh, wkl),
                        (ehtsl, wkh),
                    )):
                        nc.tensor.matmul(
                            ps_s[:], lh[:, 4 * k : 4 * k + 4], rh[:],
                            start=(k == 0 and i_ == 0), stop=(k == 3 and i_ == 2),
                        )
            s_h = p0c.tile([4, 512], bf16, tag="s_h")
            s_l = p0c.tile([4, 512], bf16, tag="s_l")
            nc.vector.tensor_scalar_mul(s_h[:], ps_s[:], 0.5)
            nc.vector.scalar_tensor_tensor(
                s_l[:], ps_s[:], 0.5, s_h[:], op0=OP.mult, op1=OP.subtract
            )
            stTh = p0c.tile([128, 16], bf16, tag="stTh")
            stTl = p0c.tile([128, 16], bf16, tag="stTl")
            for s_sb, stT in ((s_h, stTh), (s_l, stTl)):
                for j in range(4):
                    pt = ps0c.tile([128, 16], bf16, tag="pts")
                    nc.tensor.transpose(
                        pt[:, 4 * j : 4 * j + 4], s_sb[:, 128 * j : 128 * (j + 1)], I4b[:]
                    )
                    nc.vector.tensor_copy(stT[:, 4 * j : 4 * j + 4], pt[:, 4 * j : 4 * j + 4])

            # Wmod rows chunk mc (q-outer so W2 slices are loaded once)
            for q in range(4):
                w2qh = [p0w2.tile([128, 512], bf16, tag="w2qh", name=f"w2qh{q}_{kt}") for kt in range(4)]
                w2ql = [p0w2.tile([128, 512], bf16, tag="w2ql", name=f"w2ql{q}_{kt}") for kt in range(4)]
                for kt in range(4):
                    nc.gpsimd.dma_start(
                        w2qh[kt][:], W2h_d[128 * kt : 128 * (kt + 1), 512 * q : 512 * (q + 1)]
                    )
                    nc.gpsimd.dma_start(
                        w2ql[kt][:], W2l_d[128 * kt : 128 * (kt + 1), 512 * q : 512 * (q + 1)]
                    )
                for mc in range(8):
                    pm = ps0c.tile([128, 512], f32, tag="pm")
                    nmm = 0
                    for kt in range(4):
                        for lh, rh in (
                            (wath[kt], w2qh[kt]),
                            (wath[kt], w2ql[kt]),
                            (watl[kt], w2qh[kt]),
                        ):
                            nmm += 1
                            nc.tensor.matmul(
                                pm[:],
                                lh[:, 128 * mc : 128 * (mc + 1)],
                                rh[:],
                                start=(nmm == 1),
                                stop=(nmm == 12),
                            )
                    gs = 2.0 if q == 2 else 1.0
                    if mc < 4:
                        # h rows: (Ul chunk + Mfold), then x0.5 for H=2h
                        scl = 0.5 * gs
                        ul_t = p0cr.tile([128, 512], f32, tag="ul")
                        nc.sync.dma_start(
                            ul_t[:], Ul_d[128 * mc : 128 * (mc + 1), 512 * q : 512 * (q + 1)]
                        )
                        t1 = p0cr.tile([128, 512], f32, tag="t1")
                        nc.vector.tensor_tensor(t1[:], pm[:], ul_t[:], op=OP.add)
                        dh = wmodh[mc][:, 512 * q : 512 * (q + 1)]
                        nc.vector.tensor_scalar_mul(dh, t1[:], scl)
                        nc.vector.scalar_tensor_tensor(
                            wmodl[mc][:, 512 * q : 512 * (q + 1)],
                            t1[:], scl, dh, op0=OP.mult, op1=OP.subtract,
                        )
                    else:
                        dh = wmodch[mc - 4][:, 512 * q : 512 * (q + 1)]
                        nc.vector.tensor_scalar_mul(dh, pm[:], gs)
                        nc.vector.scalar_tensor_tensor(
                            wmodcl[mc - 4][:, 512 * q : 512 * (q + 1)],
                            pm[:], gs, dh, op0=OP.mult, op1=OP.subtract,
                        )

                # corr chunk q while w2q is resident
                pc = ps0s.tile([4, 512], f32, tag="ps_s")
                nmm = 0
                for kt in range(4):
                    for lh, rh in (
                        (stTh, w2qh[kt]),
                        (stTh, w2ql[kt]),
                        (stTl, w2qh[kt]),
                    ):
                        nmm += 1
                        nc.tensor.matmul(
                            pc[:], lh[:, 4 * kt : 4 * kt + 4], rh[:],
                            start=(nmm == 1), stop=(nmm == 12),
                        )
                gs = 2.0 if q == 2 else 1.0
                dh = corrh[:, 512 * q : 512 * (q + 1)]
                nc.vector.tensor_scalar_mul(dh, pc[:], gs)
                nc.vector.scalar_tensor_tensor(
                    corrl[:, 512 * q : 512 * (q + 1)],
                    pc[:], gs, dh, op0=OP.mult, op1=OP.subtract,
                )

        # ------------------------------------------------------------------
        # phase 0d: Mproj[b] = mem[b] @ Wmod_c  (uses mtv, then frees it)
        # ------------------------------------------------------------------
        with tc.tile_pool(name="ps0d", bufs=2, space="PSUM") as ps0d:
            for b in range(BL):
                for q in range(4):
                    pm = ps0d.tile([128, 512], f32, tag="pmd")
                    nmm = 0
                    for kt in range(4):
                        for lh, rh in (
                            (mtvh[kt], wmodch[kt]),
                            (mtvh[kt], wmodcl[kt]),
                            (mtvl[kt], wmodch[kt]),
                        ):
                            nmm += 1
                            nc.tensor.matmul(
                                pm[:],
                                lh[:, 128 * b : 128 * (b + 1)],
                                rh[:, 512 * q : 512 * (q + 1)],
                                start=(nmm == 1),
                                stop=(nmm == 12),
                            )
                    dh = mprojh[b][:, 512 * q : 512 * (q + 1)]
                    nc.vector.tensor_copy(dh, pm[:])
                    nc.vector.tensor_tensor(
                        mprojl[b][:, 512 * q : 512 * (q + 1)], pm[:], dh, op=OP.subtract
                    )
        wmodc_cm.__exit__(None, None, None)
        mtv_cm.__exit__(None, None, None)

        # ------------------------------------------------------------------
        # phase 1: the recurrence (C = 2c, H = 2h)
        # ------------------------------------------------------------------
        n_chunks = (n_steps * BL + 127) // 128
        with (
            tc.tile_pool(name="wk", bufs=1) as wk,
            tc.tile_pool(name="xwp", bufs=2) as xwp,
            tc.tile_pool(name="cst", bufs=2) as cst,
            tc.tile_pool(name="pz", bufs=1, space="PSUM") as pzp,
            tc.tile_pool(name="pat", bufs=1, space="PSUM") as patp,
            tc.tile_pool(name="ptr", bufs=3, space="PSUM") as ptrp,
        ):
            c_prev = cst.tile([BL, U], f32, tag="c")
            nc.sync.dma_start(c_prev[:], enc_c2_d[:])

            xwch = {}
            xwcl = {}

            def load_xw_chunk(c):
                rows = min(128, NTOK - 128 * c)
                tlh = xwp.tile([128, G], bf16, tag="xwch", name=f"xwch{c}")
                tll = xwp.tile([128, G], bf16, tag="xwcl", name=f"xwcl{c}")
                nc.gpsimd.dma_start(tlh[:rows, :], xwh_dram[128 * c : 128 * c + rows])
                nc.gpsimd.dma_start(tll[:rows, :], xwl_dram[128 * c : 128 * c + rows])
                xwch[c] = tlh
                xwcl[c] = tll

            load_xw_chunk(0)

            def z_head(t, pz):
                """Xw inject (+ step-1 corr) + H-part of z_t."""
                ch = (t - 1) // 32
                row = 4 * ((t - 1) % 32)
                for q in range(4):
                    zq = pz[:, 512 * q : 512 * (q + 1)]
                    nc.tensor.matmul(
                        zq, I128b[:, row : row + 4],
                        xwch[ch][:, 512 * q : 512 * (q + 1)],
                        start=True, stop=False,
                    )
                    nc.tensor.matmul(
                        zq, I128b[:, row : row + 4],
                        xwcl[ch][:, 512 * q : 512 * (q + 1)],
                        start=False, stop=False,
                    )
                    if t == 1:
                        nc.tensor.matmul(
                            zq, I4nb[:], corrh[:, 512 * q : 512 * (q + 1)],
                            start=False, stop=False,
                        )
                        nc.tensor.matmul(
                            zq, I4nb[:], corrl[:, 512 * q : 512 * (q + 1)],
                            start=False, stop=False,
                        )
                for kt in range(4):
                    for i_, (hh, ww) in enumerate((
                        (htallh, wmodh),
                        (htallh, wmodl),
                        (htalll, wmodh),
                    )):
                        for q in range(4):
                            nc.tensor.matmul(
                                pz[:, 512 * q : 512 * (q + 1)],
                                hT_cols(hh, kt, t - 1, 4),
                                ww[kt][:, 512 * q : 512 * (q + 1)],
                                start=False,
                                stop=(t == 1 and kt == 3 and i_ == 2),
                            )

            def z_tail(t, pz):
                """ctx contribution via alpha_{t-1} @ Mproj[b]."""
                ec = 16 * (t - 2)
                for b in range(BL):
                    last_b = b == BL - 1
                    for i_, (ee, mm) in enumerate((
                        (eTallh, mprojh),
                        (eTallh, mprojl),
                        (eTalll, mprojh),
                    )):
                        for q in range(4):
                            nc.tensor.matmul(
                                pz[:, 512 * q : 512 * (q + 1)],
                                ee[:, ec + 4 * b : ec + 4 * b + 4],
                                mm[b][:, 512 * q : 512 * (q + 1)],
                                start=False,
                                stop=(last_b and i_ == 2),
                            )

            pz_cur = pzp.tile([BL, G], f32, tag="pz", name="pz1")
            z_head(1, pz_cur)

            for t in range(1, n_steps + 1):
                if t % 32 == 2 and (t - 1) // 32 + 1 < n_chunks:
                    load_xw_chunk((t - 1) // 32 + 1)

                pz = pz_cur

                # --- gates (tanh table only, doubled states) ---
                th = wk.tile([BL, G], f32, tag="th")
                nc.scalar.activation(th[:], pz[:], AF.Tanh, bias=0.0, scale=0.5)

                # score-pairs mask contribution: no step deps
                psc = patp.tile([BL, BL * T], f32, tag="pat")
                nc.tensor.matmul(psc[:], I4b[:], mnegb[:], start=True, stop=False)

                # u = (th_f+1)*C ; v = (th_i+1)*th_g ; C' = 0.5u + v
                u = wk.tile([BL, U], f32, tag="u")
                v = wk.tile([BL, U], f32, tag="v")
                nc.vector.scalar_tensor_tensor(
                    u[:], th[:, 512:1024], 1.0, c_prev[:], op0=OP.add, op1=OP.mult
                )
                nc.vector.scalar_tensor_tensor(
                    v[:], th[:, 0:512], 1.0, th[:, 1024:1536], op0=OP.add, op1=OP.mult
                )
                c_new = cst.tile([BL, U], f32, tag="c")
                nc.vector.scalar_tensor_tensor(
                    c_new[:], u[:], 0.5, v[:], op0=OP.mult, op1=OP.add
                )
                tc_ = wk.tile([BL, U], f32, tag="tc")
                nc.scalar.activation(tc_[:], c_new[:], AF.Tanh, bias=0.0, scale=0.5)
                h = wk.tile([BL, U], f32, tag="h")  # H = 2h
                nc.vector.scalar_tensor_tensor(
                    h[:], th[:, 1536:2048], 1.0, tc_[:], op0=OP.add, op1=OP.mult
                )
                c_prev = c_new

                # --- hT via PE transposes -> htall hi/lo cols 4t ---
                pht = ptrp.tile([128, 16], f32, tag="ptr")
                for j in range(4):
                    nc.tensor.transpose(
                        pht[:, 4 * j : 4 * j + 4], h[:, 128 * j : 128 * (j + 1)], I4[:]
                    )
                dsth = htallh[:].rearrange("p (j s) -> p j s", j=4)[:, :, 4 * t : 4 * t + 4]
                dstl = htalll[:].rearrange("p (j s) -> p j s", j=4)[:, :, 4 * t : 4 * t + 4]
                phtv = pht[:].rearrange("p (j b) -> p j b", j=4)
                nc.vector.tensor_copy(dsth, phtv)
                nc.vector.tensor_tensor(dstl, phtv, dsth, op=OP.subtract)

                # --- score pairs [b, (b', t')] (mask already in psc) ---
                nmm = 0
                for kt in range(4):
                    for hh, kk in (
                        (htallh, kTh),
                        (htallh, kTl),
                        (htalll, kTh),
                    ):
                        nmm += 1
                        nc.tensor.matmul(
                            psc[:],
                            hT_cols(hh, kt, t, 4),
                            kk[kt][:],
                            start=False,
                            stop=(nmm == 12),
                        )

                # --- z_{t+1} head: fills the PE while softmax runs ---
                if t < n_steps:
                    pz_cur = pzp.tile([BL, G], f32, tag="pz", name=f"pz{t + 1}")
                    z_head(t + 1, pz_cur)

                # --- masked softmax straight off PSUM ---
                nmax = wk.tile([BL, 1], f32, tag="nmax")
                nc.vector.tensor_reduce(
                    nmax[:], psc[:], axis=AX.X, op=OP.max, negate=True
                )
                e = wk.tile([BL, BL * T], f32, tag="e")
                ssum = wk.tile([BL, 1], f32, tag="ssum")
                nc.scalar.activation(
                    e[:], psc[:], AF.Exp, bias=nmax[:, :1], scale=1.0,
                    accum_out=ssum[:, :1],
                )
                rec = wk.tile([BL, 1], f32, tag="rec")
                nc.vector.reciprocal(rec[:], ssum[:])
                e2 = wk.tile([BL, BL * T], f32, tag="e2")
                nc.vector.tensor_scalar(e2[:], e[:], rec[:, :1], None, op0=OP.mult)

                # --- eT blocks -> eTall hi/lo ---
                pet = ptrp.tile([128, 16], f32, tag="ptr")
                for q in range(BL):
                    nc.tensor.transpose(
                        pet[:, 4 * q : 4 * q + 4], e2[:, T * q : T * (q + 1)], I4[:]
                    )
                eh = eTallh[:, 16 * (t - 1) : 16 * t]
                nc.vector.tensor_copy(eh, pet[:])
                nc.vector.tensor_tensor(
                    eTalll[:, 16 * (t - 1) : 16 * t], pet[:], eh, op=OP.subtract
                )

                # --- z_{t+1} tail: ctx contribution, needs eTall of step t ---
                if t < n_steps:
                    z_tail(t + 1, pz_cur)

        mproj_cm.__exit__(None, None, None)

        # ------------------------------------------------------------------
        # phase 2 (single bf16): ctxT from eTall_hi; attn = [H|CTX] @ Wa_att;
        # logits = attn @ Wfc
        # ------------------------------------------------------------------
        with (
            tc.tile_pool(name="p2", bufs=1) as p2,
            tc.tile_pool(name="p2r", bufs=3) as p2r,
            tc.tile_pool(name="ps2", bufs=4, space="PSUM") as ps2,
        ):
            ntok = BL * n_steps

            # ctxT: ct2[j][:, 4*k + b] = ctx_{k+1}[b, 128j + u']
            ct2 = [p2.tile([128, NTOK], bf16, tag=f"ct2{j}", name=f"ct2{j}") for j in range(4)]
            eview = eTallh[:].rearrange("p (t s) -> p t s", s=16)
            for j in range(4):
                for b in range(BL):
                    pc2 = ps2.tile([128, T], f32, tag="pc2", bufs=2)
                    nc.tensor.matmul(
                        pc2[:, :n_steps],
                        mpack[:, U * b + 128 * j : U * b + 128 * (j + 1)],
                        eview[:, 0:n_steps, 4 * b + b],
                        start=True,
                        stop=True,
                    )
                    dst = ct2[j][:].rearrange("p (k b) -> p k b", b=4)[:, 0:n_steps, b]
                    nc.vector.tensor_copy(dst, pc2[:, :n_steps])

            wax = [p2.tile([128, U], bf16, tag=f"wax{k}", name=f"wax{k}") for k in range(8)]
            for k in range(8):
                nc.gpsimd.dma_start(wax[k][:], Waatt_d[128 * k : 128 * (k + 1)])
            att = [p2.tile([128, NTOK], bf16, tag=f"att{j}", name=f"att{j}") for j in range(4)]
            for j in range(4):
                pa = ps2.tile([128, 512], f32, tag="pa", bufs=2)
                for kt in range(8):
                    if kt < 4:
                        src = htallh[:].rearrange("p (jj s) -> p jj s", jj=4)[
                            :, kt, 4 : 4 + ntok
                        ]
                    else:
                        src = ct2[kt - 4][:, :ntok]
                    nc.tensor.matmul(
                        pa[:, :ntok],
                        wax[kt][:, 128 * j : 128 * (j + 1)],
                        src,
                        start=(kt == 0),
                        stop=(kt == 7),
                    )
                nc.vector.tensor_copy(att[j][:, :ntok], pa[:, :ntok])

            NCH = (VO + 511) // 512  # 63
            for nci in range(NCH):
                # last chunk overlaps the previous one so every chunk is a
                # full 512 wide
                n0 = min(512 * nci, VO - 512)
                ncols = 512
                wf = p2r.tile([128, 4, 512], bf16, tag="wf")
                nc.gpsimd.dma_start(
                    wf[:, :, :ncols],
                    Wfc_d[:, n0 : n0 + ncols].rearrange("(k p) n -> p k n", k=4),
                )
                for mt in range((ntok + 127) // 128):
                    mrows = min(128, ntok - 128 * mt)
                    pl = ps2.tile([128, 512], f32, tag="pl", bufs=4)
                    for kt in range(4):
                        nc.tensor.matmul(
                            pl[:mrows, :ncols],
                            att[kt][:, 128 * mt : 128 * mt + mrows],
                            wf[:, kt, :ncols],
                            start=(kt == 0),
                            stop=(kt == 3),
                        )
                    ot = p2r.tile([128, 512], bf16, tag="ot")
                    nc.vector.tensor_copy(ot[:mrows, :ncols], pl[:mrows, :ncols])
                    nc.sync.dma_start(
                        out_d[128 * mt : 128 * mt + mrows, n0 : n0 + ncols],
                        ot[:mrows, :ncols],
                    )

        dram_cm.__exit__(None, None, None)
        per_cm.__exit__(None, None, None)

    nc.compile()
    return nc


def _bf16_split(a):
    import ml_dtypes

    a = np.ascontiguousarray(a, np.float32)
    hi = a.astype(ml_dtypes.bfloat16)
    lo = (a - hi.astype(np.float32)).astype(ml_dtypes.bfloat16)
    return np.ascontiguousarray(hi), np.ascontiguousarray(lo)


def _shard_inputs(inputs, memory, enc_h, enc_c, E, Wm, W_lstm, U_lstm, b_lstm, Wa, Wfc, bfc):
    import ml_dtypes

    inputs = np.ascontiguousarray(inputs)
    Eh, El = _bf16_split(E)
    Wmh, Wml = _bf16_split(Wm)
    W1h, W1l = _bf16_split(W_lstm[:D])
    W2h, W2l = _bf16_split(W_lstm[D:])
    blh, bll = _bf16_split(b_lstm.reshape(1, G))
    Wah, Wal = _bf16_split(Wa)
    # phase-2 attention Wa: h rows pre-halved (H=2h), single bf16
    Wa_att = np.ascontiguousarray(Wa, np.float32).copy()
    Wa_att[:U] *= 0.5
    shared = {
        "Eh": Eh, "El": El,
        "Wmh": Wmh, "Wml": Wml,
        "W1h": W1h, "W1l": W1l,
        "W2h": W2h, "W2l": W2l,
        "Ul": np.ascontiguousarray(U_lstm, np.float32),
        "blh": blh, "bll": bll,
        "Wah": Wah, "Wal": Wal,
        "Waatt": Wa_att.astype(ml_dtypes.bfloat16),
        "Wfc": np.ascontiguousarray(Wfc, np.float32).astype(ml_dtypes.bfloat16),
    }
    in_maps = []
    for rk in range(NCORES):
        sl = slice(BL * rk, BL * (rk + 1))
        m = dict(shared)
        m["tok_ids"] = np.ascontiguousarray(inputs[sl].T.reshape(NTOK, 1), np.int32)
        m["mem"] = np.ascontiguousarray(memory[sl], np.float32)
        hth, htl = _bf16_split(2.0 * enc_h[sl].T)
        m["enc_hth"] = hth
        m["enc_htl"] = htl
        m["enc_c2"] = np.ascontiguousarray(2.0 * enc_c[sl], np.float32)
        in_maps.append(m)
    return in_maps


def kernel(**inputs):
    from concourse.bass_utils import run_bass_kernel_spmd

    if "nc" not in _cache:
        _cache["nc"] = _build(T)
    nc = _cache["nc"]

    in_maps = _shard_inputs(**inputs)
    res = run_bass_kernel_spmd(nc, in_maps, core_ids=list(range(NCORES)))
    outs = []
    for rk in range(NCORES):
        o = np.asarray(res.results[rk]["out"], dtype=np.float32)  # [512, 32001]
        outs.append(o.reshape(T, BL, VO).transpose(1, 0, 2))
    full = np.concatenate(outs, axis=0)  # [32, 128, 32001]
    full += np.asarray(inputs["bfc"], np.float32)[None, None, :]
    return full


# revision 29
# speedup vs baseline: 1.9876x; 1.3519x over previous
"""Trainium2 Bass kernel for nn_Decoder (LSTM decoder + Luong attention + vocab proj).

Strategy (8 cores, data-parallel over batch, B_local = 4):
  All recurrence-feeding matmuls run as 3-term bf16 hi/lo splits
  (a@b ~= ah@bh + ah@bl + al@bh, each 1 cyc/row on the PE vs 4 for fp32;
  dropped al@bl term ~2^-16 relative, well inside the 2e-2 tolerance as
  the chaotic recurrence amplifies per-step noise ~1e4x).

  phase 0: on-device prep per core (bf16-split GEMMs):
    - embedding gather (indirect DMA on host-split E_hi/E_lo) +
      Xw = X @ W1 + bl for all steps, kept in SBUF as bf16 hi/lo chunks
    - kT = 0.5*(mem @ Wm)^T per batch (0.5 folds the doubled-h state)
    - Wmod = [0.5*(Ul + Wa_h @ W2) ; Wa_c @ W2] (g cols x2 pre-scaling),
      so per step z_t = Xw_t + H @ Wmod_h + alpha @ Mproj, H = 2h
    - Mproj[b] = mem[b] @ Wmod_c
    - corr = h0 @ (Wa_h @ W2) step-1 correction (attn_0 = 0)
  phase 1: 128 sequential steps with doubled states C=2c, H=2h:
    gates need only tanh tables: u=(th_f+1)*C, v=(th_i+1)*th_g,
    C'=0.5u+v, H=(th_o+1)*tanh(0.5C') - 4 fused DVE ops + 2 ACT.
    score via all-pairs matmul with additive block-diag mask; softmax
    weights transposed into eTall (bf16 hi/lo) for the next z and phase 2.
  phase 2 (single bf16, feeds only the final logits): ctx from eTall_hi,
    attn = [H|CTX] @ Wa_att (h rows pre-halved), logits = attn @ Wfc(bf16)
    streamed from HBM, output written bf16 (host upcasts + adds bfc).
"""

import sys

for _p in ("/opt/trn_rl_repo",):
    if _p not in sys.path:
        sys.path.insert(0, _p)

import numpy as np

B, T, V, D, U = 32, 128, 32000, 256, 512
VO = V + 1
NCORES = 8
BL = B // NCORES  # 4
G = 4 * U  # 2048
NTOK = BL * T  # 512 tokens per core
HT_W = 4 * (T + 1)  # 516 columns per u-chunk in hT buffer

_cache = {}


def _build(n_steps=T):
    import concourse.bacc as bacc
    import concourse.bass as bass
    import concourse.mybir as mybir
    import concourse.tile as tile
    from concourse.masks import make_identity

    f32 = mybir.dt.float32
    bf16 = mybir.dt.bfloat16
    i32 = mybir.dt.int32
    AX = mybir.AxisListType
    OP = mybir.AluOpType
    AF = mybir.ActivationFunctionType

    try:
        import concourse.tile_utils as _tu

        if getattr(_tu, "max_sbuf_usage", 0) < 204 * 1024:
            _tu.max_sbuf_usage = 204 * 1024
    except Exception:
        pass

    nc = bacc.Bacc(None, target_bir_lowering=False)

    tok_ids = nc.dram_tensor("tok_ids", [NTOK, 1], i32, kind="ExternalInput")
    mem_d = nc.dram_tensor("mem", [BL, T, U], f32, kind="ExternalInput")
    # 2*enc_h transposed, bf16 hi/lo
    enc_hth_d = nc.dram_tensor("enc_hth", [U, BL], bf16, kind="ExternalInput")
    enc_htl_d = nc.dram_tensor("enc_htl", [U, BL], bf16, kind="ExternalInput")
    enc_c2_d = nc.dram_tensor("enc_c2", [BL, U], f32, kind="ExternalInput")
    Eh_d = nc.dram_tensor("Eh", [V, D], bf16, kind="ExternalInput")
    El_d = nc.dram_tensor("El", [V, D], bf16, kind="ExternalInput")
    Wmh_d = nc.dram_tensor("Wmh", [U, U], bf16, kind="ExternalInput")
    Wml_d = nc.dram_tensor("Wml", [U, U], bf16, kind="ExternalInput")
    W1h_d = nc.dram_tensor("W1h", [D, G], bf16, kind="ExternalInput")
    W1l_d = nc.dram_tensor("W1l", [D, G], bf16, kind="ExternalInput")
    W2h_d = nc.dram_tensor("W2h", [U, G], bf16, kind="ExternalInput")
    W2l_d = nc.dram_tensor("W2l", [U, G], bf16, kind="ExternalInput")
    Ul_d = nc.dram_tensor("Ul", [U, G], f32, kind="ExternalInput")
    blh_d = nc.dram_tensor("blh", [1, G], bf16, kind="ExternalInput")
    bll_d = nc.dram_tensor("bll", [1, G], bf16, kind="ExternalInput")
    Wah_d = nc.dram_tensor("Wah", [2 * U, U], bf16, kind="ExternalInput")
    Wal_d = nc.dram_tensor("Wal", [2 * U, U], bf16, kind="ExternalInput")
    Waatt_d = nc.dram_tensor("Waatt", [2 * U, U], bf16, kind="ExternalInput")
    Wfc_d = nc.dram_tensor("Wfc", [U, VO], bf16, kind="ExternalInput")
    out_d = nc.dram_tensor("out", [NTOK, VO], bf16, kind="ExternalOutput")

    with tile.TileContext(nc) as tc:
        # ------------------------------------------------------------------
        # persistent pool
        # ------------------------------------------------------------------
        per_cm = tc.tile_pool(name="per", bufs=1)
        per = per_cm.__enter__()

        wmodh = [per.tile([128, G], bf16, tag=f"wmodh{k}", name=f"wmodh{k}") for k in range(4)]
        wmodl = [per.tile([128, G], bf16, tag=f"wmodl{k}", name=f"wmodl{k}") for k in range(4)]
        kTh = [per.tile([128, BL * T], bf16, tag=f"kTh{j}", name=f"kTh{j}") for j in range(4)]
        kTl = [per.tile([128, BL * T], bf16, tag=f"kTl{j}", name=f"kTl{j}") for j in range(4)]
        mpack = per.tile([128, BL * U], bf16, tag="mpack")  # [t, (b,u)] single bf16
        htallh = per.tile([128, 4 * HT_W], bf16, tag="htallh")
        htalll = per.tile([128, 4 * HT_W], bf16, tag="htalll")
        eTallh = per.tile([128, 16 * T], bf16, tag="eTallh")
        eTalll = per.tile([128, 16 * T], bf16, tag="eTalll")
        corrh = per.tile([BL, G], bf16, tag="corrh")
        corrl = per.tile([BL, G], bf16, tag="corrl")
        dram_cm = tc.tile_pool(name="dram", bufs=1, space="DRAM")
        dram = dram_cm.__enter__()
        xwh_dram = dram.tile([NTOK, G], bf16, name="xwh_dram")
        xwl_dram = dram.tile([NTOK, G], bf16, name="xwl_dram")
        I4 = per.tile([4, 4], f32, tag="I4")        # f32: transpose identity
        I4b = per.tile([4, 4], bf16, tag="I4b")     # bf16 matmul lhsT
        I4nb = per.tile([4, 4], bf16, tag="I4nb")
        I128b = per.tile([128, 128], bf16, tag="I128b")
        J2b = per.tile([128, 64], bf16, tag="J2b")  # two-hot: (j,j) and (j+64,j)
        Z4b = per.tile([4, 128], bf16, tag="Z4b")   # zero lhsT: clears a z bank
        onesb = per.tile([1, 128], bf16, tag="onesb")
        mnegb = per.tile([BL, BL * T], bf16, tag="mnegb")

        make_identity(nc, I4[:])
        nc.gpsimd.memset(Z4b[:], 0.0)
        nc.vector.tensor_copy(I4b[:], I4[:])
        nc.vector.tensor_scalar_mul(I4nb[:], I4[:], -1.0)
        with tc.tile_pool(name="pinit", bufs=1) as pinit:
            onesf = pinit.tile([1, 128], f32, tag="onesf")
            nc.gpsimd.memset(onesf[:], 1.0)
            nc.vector.tensor_copy(onesb[:], onesf[:])
            # J2[p,j] = 1 iff p == j or p == j+64 (injects xw_hi+xw_lo rows)
            j2x = pinit.tile([128, 64], f32, tag="j2x")
            nc.gpsimd.iota(
                j2x[:], pattern=[[-1, 64]], base=0, channel_multiplier=1,
                allow_small_or_imprecise_dtypes=True,
            )
            j2a = pinit.tile([128, 64], f32, tag="j2a")
            j2c = pinit.tile([128, 64], f32, tag="j2c")
            nc.vector.tensor_scalar(j2a[:], j2x[:], 0.0, None, op0=OP.is_equal)
            nc.vector.tensor_scalar(j2c[:], j2x[:], 64.0, None, op0=OP.is_equal)
            nc.vector.tensor_tensor(j2a[:], j2a[:], j2c[:], op=OP.add)
            nc.vector.tensor_copy(J2b[:], j2a[:])
            # block-diagonal additive mask: 0 on own 128-block, -1e30 elsewhere.
            miot = pinit.tile([BL, BL * T], f32, tag="miot")
            nc.gpsimd.iota(
                miot[:], pattern=[[1, BL * T]], base=0, channel_multiplier=-T,
                allow_small_or_imprecise_dtypes=True,
            )
            ma = pinit.tile([BL, BL * T], f32, tag="ma")
            mb = pinit.tile([BL, BL * T], f32, tag="mb")
            nc.vector.tensor_scalar(ma[:], miot[:], 0.0, None, op0=OP.is_ge)
            nc.vector.tensor_scalar(mb[:], miot[:], float(T - 1), None, op0=OP.is_le)
            nc.vector.tensor_tensor(ma[:], ma[:], mb[:], op=OP.mult)
            nc.vector.tensor_scalar(mnegb[:], ma[:], -1.0, 1e30, op0=OP.add, op1=OP.mult)

        # hT layout: htall[:, HT_W*j + 4*t + b] = H_t[b, 128*j + u']
        def hT_cols(tl, j, t0, ncols):
            v = tl[:].rearrange("p (j s) -> p j s", j=4)
            return v[:, j, 4 * t0 : 4 * t0 + ncols]

        # init H_0 = 2*enc_h (transposed+split on host)
        for tl, src in ((htallh, enc_hth_d), (htalll, enc_htl_d)):
            nc.gpsimd.dma_start(
                tl[:].rearrange("p (j s) -> p j s", j=4)[:, :, 0:BL],
                src[:].rearrange("(j p) b -> p j b", j=4),
            )

        # ------------------------------------------------------------------
        # phase 0a: mem transposes (mtv hi/lo + mpack), keys, embedding + Xw
        # ------------------------------------------------------------------
        mproj_cm = tc.tile_pool(name="mprojp", bufs=1)
        mprojp = mproj_cm.__enter__()
        mprojh = [mprojp.tile([128, G], bf16, tag=f"mprojh{b}", name=f"mprojh{b}") for b in range(BL)]
        mprojl = [mprojp.tile([128, G], bf16, tag=f"mprojl{b}", name=f"mprojl{b}") for b in range(BL)]
        mtv_cm = tc.tile_pool(name="mtvp", bufs=1)
        mtvp = mtv_cm.__enter__()
        mtvh = [mtvp.tile([128, BL * 128], bf16, tag=f"mtvh{v}", name=f"mtvh{v}") for v in range(4)]
        mtvl = [mtvp.tile([128, BL * 128], bf16, tag=f"mtvl{v}", name=f"mtvl{v}") for v in range(4)]

        with (
            tc.tile_pool(name="p0a", bufs=2) as p0a,
            tc.tile_pool(name="p0a1", bufs=1) as p0a1,
            tc.tile_pool(name="ps0", bufs=2, space="PSUM") as ps0,
        ):
            I128 = p0a1.tile([128, 128], f32, tag="I128")
            make_identity(nc, I128[:])
            nc.vector.tensor_copy(I128b[:], I128[:])

            # memT + mpack
            for b in range(BL):
                memf = p0a.tile([128, U], f32, tag="memf", name=f"memf{b}")
                nc.sync.dma_start(memf[:], mem_d[b])
                nc.vector.tensor_copy(mpack[:, U * b : U * (b + 1)], memf[:])
                for vc in range(4):
                    pt = ps0.tile([128, 128], f32, tag="pt0")
                    nc.tensor.transpose(
                        pt[:], memf[:, 128 * vc : 128 * (vc + 1)], I128[:]
                    )
                    dh = mtvh[vc][:, 128 * b : 128 * (b + 1)]
                    nc.vector.tensor_copy(dh, pt[:])
                    nc.vector.tensor_tensor(
                        mtvl[vc][:, 128 * b : 128 * (b + 1)], pt[:], dh, op=OP.subtract
                    )

            # kT = 0.5 * keysT (0.5 folds H=2h)
            wmsh = [p0a1.tile([128, U], bf16, tag=f"wmsh{k}", name=f"wmsh{k}") for k in range(4)]
            wmsl = [p0a1.tile([128, U], bf16, tag=f"wmsl{k}", name=f"wmsl{k}") for k in range(4)]
            for k in range(4):
                nc.gpsimd.dma_start(wmsh[k][:], Wmh_d[128 * k : 128 * (k + 1)])
                nc.gpsimd.dma_start(wmsl[k][:], Wml_d[128 * k : 128 * (k + 1)])
            for j in range(4):
                for b in range(BL):
                    pk = ps0.tile([128, 128], f32, tag="pt0")
                    nmm = 0
                    for vt in range(4):
                        for lh, rh in (
                            (wmsh[vt], mtvh[vt]),
                            (wmsh[vt], mtvl[vt]),
                            (wmsl[vt], mtvh[vt]),
                        ):
                            nmm += 1
                            nc.tensor.matmul(
                                pk[:],
                                lh[:, 128 * j : 128 * (j + 1)],
                                rh[:, 128 * b : 128 * (b + 1)],
                                start=(nmm == 1),
                                stop=(nmm == 12),
                            )
                    dh = kTh[j][:, 128 * b : 128 * (b + 1)]
                    nc.vector.tensor_scalar_mul(dh, pk[:], 0.5)
                    nc.vector.scalar_tensor_tensor(
                        kTl[j][:, 128 * b : 128 * (b + 1)],
                        pk[:], 0.5, dh, op0=OP.mult, op1=OP.subtract,
                    )

            # embedding gather (hi/lo) + transposes
            xth = [p0a1.tile([128, NTOK], bf16, tag=f"xth{k}", name=f"xth{k}") for k in range(2)]
            xtl = [p0a1.tile([128, NTOK], bf16, tag=f"xtl{k}", name=f"xtl{k}") for k in range(2)]
            for c in range(NTOK // 128):
                ids_c = p0a.tile([128, 1], i32, tag="ids")
                nc.sync.dma_start(ids_c[:], tok_ids[128 * c : 128 * (c + 1)])
                xch = p0a.tile([128, D], bf16, tag="xch")
                xcl = p0a.tile([128, D], bf16, tag="xcl")
                for xc, E_d in ((xch, Eh_d), (xcl, El_d)):
                    nc.gpsimd.indirect_dma_start(
                        out=xc[:],
                        out_offset=None,
                        in_=E_d[:],
                        in_offset=bass.IndirectOffsetOnAxis(ap=ids_c[:, :1], axis=0),
                    )
                for k in range(2):
                    for xc, xt in ((xch, xth), (xcl, xtl)):
                        pt = ps0.tile([128, 128], bf16, tag="ptb")
                        nc.tensor.transpose(
                            pt[:], xc[:, 128 * k : 128 * (k + 1)], I128b[:]
                        )
                        nc.vector.tensor_copy(xt[k][:, 128 * c : 128 * (c + 1)], pt[:])

            # Xw = X @ W1 + bl (g cols x2), kept in SBUF bf16 hi/lo
            blh = p0a1.tile([1, G], bf16, tag="blh")
            bll = p0a1.tile([1, G], bf16, tag="bll")
            nc.gpsimd.dma_start(blh[:], blh_d[:])
            nc.gpsimd.dma_start(bll[:], bll_d[:])
            for q in range(4):
                w1qh = [p0a.tile([128, 512], bf16, tag="w1qh", name=f"w1qh{q}_{k}") for k in range(2)]
                w1ql = [p0a.tile([128, 512], bf16, tag="w1ql", name=f"w1ql{q}_{k}") for k in range(2)]
                for k in range(2):
                    nc.gpsimd.dma_start(
                        w1qh[k][:], W1h_d[128 * k : 128 * (k + 1), 512 * q : 512 * (q + 1)]
                    )
                    nc.gpsimd.dma_start(
                        w1ql[k][:], W1l_d[128 * k : 128 * (k + 1), 512 * q : 512 * (q + 1)]
                    )
                for c in range(NTOK // 128):
                    pz0 = ps0.tile([128, 512], f32, tag="pz0")
                    nc.tensor.matmul(
                        pz0[:], onesb[:1, :128], blh[:1, 512 * q : 512 * (q + 1)],
                        start=True, stop=False,
                    )
                    nc.tensor.matmul(
                        pz0[:], onesb[:1, :128], bll[:1, 512 * q : 512 * (q + 1)],
                        start=False, stop=False,
                    )
                    nmm = 0
                    for k in range(2):
                        for lh, rh in (
                            (xth[k], w1qh[k]),
                            (xth[k], w1ql[k]),
                            (xtl[k], w1qh[k]),
                        ):
                            nmm += 1
                            nc.tensor.matmul(
                                pz0[:],
                                lh[:, 128 * c : 128 * (c + 1)],
                                rh[:],
                                start=False,
                                stop=(nmm == 6),
                            )
                    scl = 2.0 if q == 2 else 1.0
                    sth = p0a.tile([128, 512], bf16, tag="xwsth")
                    stl = p0a.tile([128, 512], bf16, tag="xwstl")
                    nc.vector.tensor_scalar_mul(sth[:], pz0[:], scl)
                    nc.vector.scalar_tensor_tensor(
                        stl[:], pz0[:], scl, sth[:], op0=OP.mult, op1=OP.subtract
                    )
                    nc.sync.dma_start(
                        xwh_dram[128 * c : 128 * (c + 1), 512 * q : 512 * (q + 1)], sth[:]
                    )
                    nc.sync.dma_start(
                        xwl_dram[128 * c : 128 * (c + 1), 512 * q : 512 * (q + 1)], stl[:]
                    )

        # ------------------------------------------------------------------
        # phase 0c: Wmod = [0.5(Ul + Wa_h @ W2) ; Wa_c @ W2], g cols x2; corr
        # ------------------------------------------------------------------
        wmodc_cm = tc.tile_pool(name="wmodcp", bufs=1)
        wmodcp = wmodc_cm.__enter__()
        wmodch = [wmodcp.tile([128, G], bf16, tag=f"wmodch{k}", name=f"wmodch{k}") for k in range(4)]
        wmodcl = [wmodcp.tile([128, G], bf16, tag=f"wmodcl{k}", name=f"wmodcl{k}") for k in range(4)]

        with (
            tc.tile_pool(name="p0c", bufs=1) as p0c,
            tc.tile_pool(name="p0cw", bufs=2) as p0cw,
            tc.tile_pool(name="p0cr", bufs=2) as p0cr,
            tc.tile_pool(name="p0w2", bufs=4) as p0w2,
            tc.tile_pool(name="ps0c", bufs=2, space="PSUM") as ps0c,
            tc.tile_pool(name="ps0s", bufs=1, space="PSUM") as ps0s,
        ):
            # enc_ht as lhsT tiles: ehts[:, 4*kt + b] (holds 2*h0)
            ehtsh = p0c.tile([128, 16], bf16, tag="ehtsh")
            ehtsl = p0c.tile([128, 16], bf16, tag="ehtsl")
            for tl, src in ((ehtsh, enc_hth_d), (ehtsl, enc_htl_d)):
                nc.gpsimd.dma_start(
                    tl[:].rearrange("p (k b) -> p k b", k=4),
                    src[:].rearrange("(k p) b -> p k b", k=4),
                )

            # stream Wa chunks: transposes into wat, s-accumulation for corr
            wath = [p0c.tile([128, 2 * U], bf16, tag=f"wath{q}", name=f"wath{q}") for q in range(4)]
            watl = [p0c.tile([128, 2 * U], bf16, tag=f"watl{q}", name=f"watl{q}") for q in range(4)]
            ps_s = ps0s.tile([4, 512], f32, tag="ps_s")
            for k in range(8):
                wkh = p0cw.tile([128, U], bf16, tag="wkh", name=f"wkh{k}")
                wkl = p0cw.tile([128, U], bf16, tag="wkl", name=f"wkl{k}")
                nc.gpsimd.dma_start(wkh[:], Wah_d[128 * k : 128 * (k + 1)])
                nc.gpsimd.dma_start(wkl[:], Wal_d[128 * k : 128 * (k + 1)])
                for q in range(4):
                    for was, wat in ((wkh, wath), (wkl, watl)):
                        pt = ps0c.tile([128, 128], bf16, tag="ptc")
                        nc.tensor.transpose(
                            pt[:], was[:, 128 * q : 128 * (q + 1)], I128b[:]
                        )
                        # bf16-valued, so the copy is exact
                        nc.vector.tensor_copy(wat[q][:, 128 * k : 128 * (k + 1)], pt[:])
                if k < 4:
                    # corr s-part: s = h0 @ Wa_h (0.5 un-doubles h0)
                    for i_, (lh, rh) in enumerate((
                        (ehtsh, wkh),
                        (ehtsh, wkl),
                        (ehtsl, wkh),
                    )):
                        nc.tensor.matmul(
                            ps_s[:], lh[:, 4 * k : 4 * k + 4], rh[:],
                            start=(k == 0 and i_ == 0), stop=(k == 3 and i_ == 2),
                        )
            s_h = p0c.tile([4, 512], bf16, tag="s_h")
            s_l = p0c.tile([4, 512], bf16, tag="s_l")
            nc.vector.tensor_scalar_mul(s_h[:], ps_s[:], 0.5)
            nc.vector.scalar_tensor_tensor(
                s_l[:], ps_s[:], 0.5, s_h[:], op0=OP.mult, op1=OP.subtract
            )
            stTh = p0c.tile([128, 16], bf16, tag="stTh")
            stTl = p0c.tile([128, 16], bf16, tag="stTl")
            for s_sb, stT in ((s_h, stTh), (s_l, stTl)):
                for j in range(4):
                    pt = ps0c.tile([128, 16], bf16, tag="pts")
                    nc.tensor.transpose(
                        pt[:, 4 * j : 4 * j + 4], s_sb[:, 128 * j : 128 * (j + 1)], I4b[:]
                    )
                    nc.vector.tensor_copy(stT[:, 4 * j : 4 * j + 4], pt[:, 4 * j : 4 * j + 4])

            # Wmod rows chunk mc (q-outer so W2 slices are loaded once)
            for q in range(4):
                w2qh = [p0w2.tile([128, 512], bf16, tag="w2qh", name=f"w2qh{q}_{kt}") for kt in range(4)]
                w2ql = [p0w2.tile([128, 512], bf16, tag="w2ql", name=f"w2ql{q}_{kt}") for kt in range(4)]
                for kt in range(4):
                    nc.gpsimd.dma_start(
                        w2qh[kt][:], W2h_d[128 * kt : 128 * (kt + 1), 512 * q : 512 * (q + 1)]
                    )
                    nc.gpsimd.dma_start(
                        w2ql[kt][:], W2l_d[128 * kt : 128 * (kt + 1), 512 * q : 512 * (q + 1)]
                    )
                for mc in range(8):
                    pm = ps0c.tile([128, 512], f32, tag="pm")
                    nmm = 0
                    for kt in range(4):
                        for lh, rh in (
                            (wath[kt], w2qh[kt]),
                            (wath[kt], w2ql[kt]),
                            (watl[kt], w2qh[kt]),
                        ):
                            nmm += 1
                            nc.tensor.matmul(
                                pm[:],
                                lh[:, 128 * mc : 128 * (mc + 1)],
                                rh[:],
                                start=(nmm == 1),
                                stop=(nmm == 12),
                            )
                    gs = 2.0 if q == 2 else 1.0
                    if mc < 4:
                        # h rows: (Ul chunk + Mfold), then x0.5 for H=2h
                        scl = 0.5 * gs
                        ul_t = p0cr.tile([128, 512], f32, tag="ul")
                        nc.sync.dma_start(
                            ul_t[:], Ul_d[128 * mc : 128 * (mc + 1), 512 * q : 512 * (q + 1)]
                        )
                        t1 = p0cr.tile([128, 512], f32, tag="t1")
                        nc.vector.tensor_tensor(t1[:], pm[:], ul_t[:], op=OP.add)
                        dh = wmodh[mc][:, 512 * q : 512 * (q + 1)]
                        nc.vector.tensor_scalar_mul(dh, t1[:], scl)
                        nc.vector.scalar_tensor_tensor(
                            wmodl[mc][:, 512 * q : 512 * (q + 1)],
                            t1[:], scl, dh, op0=OP.mult, op1=OP.subtract,
                        )
                    else:
                        dh = wmodch[mc - 4][:, 512 * q : 512 * (q + 1)]
                        nc.vector.tensor_scalar_mul(dh, pm[:], gs)
                        nc.vector.scalar_tensor_tensor(
                            wmodcl[mc - 4][:, 512 * q : 512 * (q + 1)],
                            pm[:], gs, dh, op0=OP.mult, op1=OP.subtract,
                        )

                # corr chunk q while w2q is resident
                pc = ps0s.tile([4, 512], f32, tag="ps_s")
                nmm = 0
                for kt in range(4):
                    for lh, rh in (
                        (stTh, w2qh[kt]),
                        (stTh, w2ql[kt]),
                        (stTl, w2qh[kt]),
                    ):
                        nmm += 1
                        nc.tensor.matmul(
                            pc[:], lh[:, 4 * kt : 4 * kt + 4], rh[:],
                            start=(nmm == 1), stop=(nmm == 12),
                        )
                gs = 2.0 if q == 2 else 1.0
                dh = corrh[:, 512 * q : 512 * (q + 1)]
                nc.vector.tensor_scalar_mul(dh, pc[:], gs)
                nc.vector.scalar_tensor_tensor(
                    corrl[:, 512 * q : 512 * (q + 1)],
                    pc[:], gs, dh, op0=OP.mult, op1=OP.subtract,
                )

        # ------------------------------------------------------------------
        # phase 0d: Mproj[b] = mem[b] @ Wmod_c  (uses mtv, then frees it)
        # ------------------------------------------------------------------
        with tc.tile_pool(name="ps0d", bufs=2, space="PSUM") as ps0d:
            for b in range(BL):
                for q in range(4):
                    pm = ps0d.tile([128, 512], f32, tag="pmd")
                    nmm = 0
                    for kt in range(4):
                        for lh, rh in (
                            (mtvh[kt], wmodch[kt]),
                            (mtvh[kt], wmodcl[kt]),
                            (mtvl[kt], wmodch[kt]),
                        ):
                            nmm += 1
                            nc.tensor.matmul(
                                pm[:],
                                lh[:, 128 * b : 128 * (b + 1)],
                                rh[:, 512 * q : 512 * (q + 1)],
                                start=(nmm == 1),
                                stop=(nmm == 12),
                            )
                    dh = mprojh[b][:, 512 * q : 512 * (q + 1)]
                    nc.vector.tensor_copy(dh, pm[:])
                    nc.vector.tensor_tensor(
                        mprojl[b][:, 512 * q : 512 * (q + 1)], pm[:], dh, op=OP.subtract
                    )
        wmodc_cm.__exit__(None, None, None)
        mtv_cm.__exit__(None, None, None)

        # ------------------------------------------------------------------
        # phase 1: the recurrence (C = 2c, H = 2h)
        # ------------------------------------------------------------------
        # z PSUM layout: [128, 512], gate q on partitions 32q..32q+3
        n_chunks = (n_steps * BL + 63) // 64
        with (
            tc.tile_pool(name="wk", bufs=1) as wk,
            tc.tile_pool(name="xwp", bufs=2) as xwp,
            tc.tile_pool(name="cst", bufs=2) as cst,
            tc.tile_pool(name="waxp", bufs=1) as waxp,
            tc.tile_pool(name="p2i", bufs=2) as p2i,
            tc.tile_pool(name="p2w", bufs=2) as p2w,
            tc.tile_pool(name="pz", bufs=2, space="PSUM") as pzp,
            tc.tile_pool(name="pat", bufs=1, space="PSUM") as patp,
            tc.tile_pool(name="ptr", bufs=1, space="PSUM") as ptrp,
            tc.tile_pool(name="ps2i", bufs=2, space="PSUM") as ps2i,
        ):
            c_prev = cst.tile([BL, U], f32, tag="c")
            nc.sync.dma_start(c_prev[:], enc_c2_d[:])

            xwc = {}

            def load_xw_chunk(c):
                # 64 tokens per chunk: rows 0:64 = hi, rows 64:128 = lo
                rows = min(64, NTOK - 64 * c)
                tl = xwp.tile([128, G], bf16, tag="xwc", name=f"xwc{c}")
                nc.gpsimd.dma_start(tl[:rows, :], xwh_dram[64 * c : 64 * c + rows])
                nc.gpsimd.dma_start(tl[64 : 64 + rows, :], xwl_dram[64 * c : 64 * c + rows])
                xwc[c] = tl

            load_xw_chunk(0)
            load_xw_chunk(1)

            # ---- interleaved phase-2 (logits) machinery ----
            NCH = (VO + 511) // 512  # 63
            ntok_all = BL * n_steps
            nch2 = (ntok_all + 127) // 128
            wax = [waxp.tile([128, U], bf16, tag=f"wax{k}", name=f"wax{k}") for k in range(8)]
            for k in range(8):
                nc.gpsimd.dma_start(wax[k][:], Waatt_d[128 * k : 128 * (k + 1)])
            eview = eTallh[:].rearrange("p (t s) -> p t s", s=16)
            hview = htallh[:].rearrange("p (j s) -> p j s", j=4)
            ct_tiles = {}
            att_tiles = {}

            def emit_ct2(k):
                tk = min(128, ntok_all - 128 * k)
                nst = tk // BL
                s0 = 32 * k
                ct = [p2i.tile([128, 128], bf16, tag=f"ct{j}", name=f"ct{k}_{j}") for j in range(4)]
                for j in range(4):
                    for b in range(BL):
                        pc = ps2i.tile([128, 512], f32, tag="p2i")
                        nc.tensor.matmul(
                            pc[:, :nst],
                            mpack[:, U * b + 128 * j : U * b + 128 * (j + 1)],
                            eview[:, s0 : s0 + nst, 4 * b + b],
                            start=True, stop=True,
                        )
                        dst = ct[j][:].rearrange("p (s b) -> p s b", b=4)[:, 0:nst, b]
                        nc.vector.tensor_copy(dst, pc[:, :nst])
                ct_tiles[k] = ct

            def emit_att(k):
                tk = min(128, ntok_all - 128 * k)
                at = [p2i.tile([128, 128], bf16, tag=f"at{j}", name=f"at{k}_{j}") for j in range(4)]
                for j in range(4):
                    pa = ps2i.tile([128, 512], f32, tag="p2i")
                    for kt in range(8):
                        if kt < 4:
                            src_ = hview[:, kt, 4 + 128 * k : 4 + 128 * k + tk]
                        else:
                            src_ = ct_tiles[k][kt - 4][:, :tk]
                        nc.tensor.matmul(
                            pa[:, :tk], wax[kt][:, 128 * j : 128 * (j + 1)], src_,
                            start=(kt == 0), stop=(kt == 7),
                        )
                    nc.vector.tensor_copy(at[j][:, :tk], pa[:, :tk])
                att_tiles[k] = at

            def emit_nci(k, nci):
                tk = min(128, ntok_all - 128 * k)
                n0 = min(512 * nci, VO - 512)
                wf = p2w.tile([128, 4, 512], bf16, tag="wf")
                nc.gpsimd.dma_start(
                    wf[:], Wfc_d[:, n0 : n0 + 512].rearrange("(kk p) n -> p kk n", kk=4)
                )
                pl = ps2i.tile([128, 512], f32, tag="p2i")
                for kt in range(4):
                    nc.tensor.matmul(
                        pl[:tk, :], att_tiles[k][kt][:, :tk], wf[:, kt, :],
                        start=(kt == 0), stop=(kt == 3),
                    )
                ot = p2w.tile([128, 512], bf16, tag="ot")
                nc.vector.tensor_copy(ot[:tk, :], pl[:tk, :])
                nc.sync.dma_start(
                    out_d[128 * k : 128 * k + tk, n0 : n0 + 512], ot[:tk, :]
                )

            def emit_unit(u_):
                if u_[0] == "ct2":
                    emit_ct2(u_[1])
                elif u_[0] == "att":
                    emit_att(u_[1])
                else:
                    emit_nci(u_[1], u_[2])

            sched = {}
            tail_units = []
            for k in range(nch2):
                tk = min(128, ntok_all - 128 * k)
                ready = (128 * k + tk) // BL
                if tk == 128 and ready + 2 <= n_steps:
                    sched.setdefault(ready + 1, []).append(("ct2", k))
                    sched.setdefault(ready + 2, []).append(("att", k))
                    step = ready + 3
                    cnt = 0
                    for nci in range(NCH):
                        if step > n_steps:
                            tail_units.append(("nci", k, nci))
                        else:
                            sched.setdefault(step, []).append(("nci", k, nci))
                            cnt += 1
                            if cnt % 2 == 0:
                                step += 1
                else:
                    tail_units.append(("ct2", k))
                    tail_units.append(("att", k))
                    tail_units += [("nci", k, nci) for nci in range(NCH)]

            # gate placement: pzA holds f@0, i@64; pzB holds o@0, g@64
            # (pairs the SBUF operands of each DVE gate op at equal bases)
            _gate_slot = {0: (0, 64), 1: (0, 0), 2: (1, 64), 3: (1, 0)}

            def zrow(pz, q):
                tl_i, off = _gate_slot[q]
                return pz[tl_i][off : off + 4, :]

            def z_inject(t, pz):
                """Zero-fill both z banks (full-tile write opens the
                accumulation), then Xw inject via two-hot J2 (+ step-1 corr)."""
                ch = (t - 1) // 16
                row = 4 * ((t - 1) % 16)
                for tl in pz:
                    nc.tensor.matmul(
                        tl[:], Z4b[:], mnegb[:], start=True, stop=False
                    )
                for q in range(4):
                    nc.tensor.matmul(
                        zrow(pz, q), J2b[:, row : row + 4],
                        xwc[ch][:, 512 * q : 512 * (q + 1)],
                        start=False, stop=False,
                    )
                    if t == 1:
                        nc.tensor.matmul(
                            zrow(pz, q), I4nb[:], corrh[:, 512 * q : 512 * (q + 1)],
                            start=False, stop=False,
                        )
                        nc.tensor.matmul(
                            zrow(pz, q), I4nb[:], corrl[:, 512 * q : 512 * (q + 1)],
                            start=False, stop=False,
                        )

            def z_close(pz):
                """Full-tile zero-add closes each bank's accumulation group."""
                for tl in pz:
                    nc.tensor.matmul(
                        tl[:], Z4b[:], mnegb[:], start=False, stop=True
                    )

            def z_hpart(t, pz, close=False):
                for kt in range(4):
                    for i_, (hh, ww) in enumerate((
                        (htallh, wmodh),
                        (htallh, wmodl),
                        (htalll, wmodh),
                    )):
                        for q in range(4):
                            nc.tensor.matmul(
                                zrow(pz, q),
                                hT_cols(hh, kt, t - 1, 4),
                                ww[kt][:, 512 * q : 512 * (q + 1)],
                                start=False,
                                stop=False,
                            )
                if close:
                    z_close(pz)

            def z_tail(t, pz):
                """ctx contribution via alpha_{t-1} @ Mproj[b]."""
                ec = 16 * (t - 2)
                for b in range(BL):
                    last_b = b == BL - 1
                    for i_, (ee, mm) in enumerate((
                        (eTallh, mprojh),
                        (eTallh, mprojl),
                        (eTalll, mprojh),
                    )):
                        for q in range(4):
                            nc.tensor.matmul(
                                zrow(pz, q),
                                ee[:, ec + 4 * b : ec + 4 * b + 4],
                                mm[b][:, 512 * q : 512 * (q + 1)],
                                start=False,
                                stop=False,
                            )
                z_close(pz)

            def mask_inject(psc):
                nc.tensor.matmul(psc[:], I4b[:], mnegb[:], start=True, stop=False)

            pz_cur = (
                pzp.tile([128, 512], f32, tag="pza", name="pza1"),
                pzp.tile([128, 512], f32, tag="pzb", name="pzb1"),
            )
            z_inject(1, pz_cur)
            z_hpart(1, pz_cur, close=True)

            for t in range(1, n_steps + 1):
                if t % 16 == 3 and (t - 1) // 16 + 2 < n_chunks:
                    load_xw_chunk((t - 1) // 16 + 2)

                pz = pz_cur

                # --- gates (tanh table only, doubled states) ---
                thA = wk.tile([128, 512], f32, tag="thA")
                thB = wk.tile([128, 512], f32, tag="thB")
                nc.scalar.activation(thA[:], pz[0][:], AF.Tanh, bias=0.0, scale=0.5)
                nc.scalar.activation(thB[:], pz[1][:], AF.Tanh, bias=0.0, scale=0.5)

                # --- early PE work for t+1 that needs nothing from step t ---
                if t < n_steps:
                    pz_cur = (
                        pzp.tile([128, 512], f32, tag="pza", name=f"pza{t + 1}"),
                        pzp.tile([128, 512], f32, tag="pzb", name=f"pzb{t + 1}"),
                    )
                    z_inject(t + 1, pz_cur)
                psc = patp.tile([BL, BL * T], f32, tag="pat", name=f"psc{t}")
                mask_inject(psc)

                # --- interleaved phase-2 filler (PE work during gate chain) ---
                for u_ in sched.get(t, []):
                    emit_unit(u_)

                # u = (th_f+1)*C ; v = (th_i+1)*th_g ; C' = 0.5u + v
                u = wk.tile([BL, U], f32, tag="u")
                v = wk.tile([BL, U], f32, tag="v")
                nc.vector.scalar_tensor_tensor(
                    u[:], thA[0:4, :], 1.0, c_prev[:], op0=OP.add, op1=OP.mult
                )
                nc.vector.scalar_tensor_tensor(
                    v[:], thA[64:68, :], 1.0, thB[64:68, :], op0=OP.add, op1=OP.mult
                )
                c_new = cst.tile([BL, U], f32, tag="c")
                nc.vector.scalar_tensor_tensor(
                    c_new[:], u[:], 0.5, v[:], op0=OP.mult, op1=OP.add
                )
                tc_ = wk.tile([BL, U], f32, tag="tc")
                nc.scalar.activation(tc_[:], c_new[:], AF.Tanh, bias=0.0, scale=0.5)
                h = wk.tile([BL, U], f32, tag="h")  # H = 2h
                nc.vector.scalar_tensor_tensor(
                    h[:], thB[0:4, :], 1.0, tc_[:], op0=OP.add, op1=OP.mult
                )
                c_prev = c_new

                # --- hT via PE transposes -> htall hi/lo cols 4t ---
                pht = ptrp.tile([128, 16], f32, tag="ptr")
                for j in range(4):
                    nc.tensor.transpose(
                        pht[:, 4 * j : 4 * j + 4], h[:, 128 * j : 128 * (j + 1)], I4[:]
                    )
                dsth = htallh[:].rearrange("p (j s) -> p j s", j=4)[:, :, 4 * t : 4 * t + 4]
                dstl = htalll[:].rearrange("p (j s) -> p j s", j=4)[:, :, 4 * t : 4 * t + 4]
                phtv = pht[:].rearrange("p (j b) -> p j b", j=4)
                nc.vector.tensor_copy(dsth, phtv)
                nc.vector.tensor_tensor(dstl, phtv, dsth, op=OP.subtract)

                # --- score pairs [b, (b', t')] (mask already in psc) ---
                nmm = 0
                for kt in range(4):
                    for hh, kk in (
                        (htallh, kTh),
                        (htallh, kTl),
                        (htalll, kTh),
                    ):
                        nmm += 1
                        nc.tensor.matmul(
                            psc[:],
                            hT_cols(hh, kt, t, 4),
                            kk[kt][:],
                            start=False,
                            stop=(nmm == 12),
                        )

                # --- z_{t+1} H-part: fills the PE while softmax runs ---
                if t < n_steps:
                    z_hpart(t + 1, pz_cur)

                # --- masked softmax straight off PSUM ---
                nmax = wk.tile([BL, 1], f32, tag="nmax")
                nc.vector.tensor_reduce(
                    nmax[:], psc[:], axis=AX.X, op=OP.max, negate=True
                )
                e = wk.tile([BL, BL * T], f32, tag="e")
                ssum = wk.tile([BL, 1], f32, tag="ssum")
                nc.scalar.activation(
                    e[:], psc[:], AF.Exp, bias=nmax[:, :1], scale=1.0,
                    accum_out=ssum[:, :1],
                )
                rec = wk.tile([BL, 1], f32, tag="rec")
                nc.vector.reciprocal(rec[:], ssum[:])
                e2 = wk.tile([BL, BL * T], f32, tag="e2")
                nc.vector.tensor_scalar(e2[:], e[:], rec[:, :1], None, op0=OP.mult)

                # --- eT blocks -> eTall hi/lo ---
                pet = ptrp.tile([128, 16], f32, tag="ptr")
                for q in range(BL):
                    nc.tensor.transpose(
                        pet[:, 4 * q : 4 * q + 4], e2[:, T * q : T * (q + 1)], I4[:]
                    )
                eh = eTallh[:, 16 * (t - 1) : 16 * t]
                nc.vector.tensor_copy(eh, pet[:])
                nc.vector.tensor_tensor(
                    eTalll[:, 16 * (t - 1) : 16 * t], pet[:], eh, op=OP.subtract
                )

                # --- z_{t+1} tail: ctx contribution, needs eTall of step t ---
                if t < n_steps:
                    z_tail(t + 1, pz_cur)

            # ---- phase-2 tail: remaining chunks / leftover col-tiles ----
            for u_ in tail_units:
                emit_unit(u_)

        mproj_cm.__exit__(None, None, None)
        dram_cm.__exit__(None, None, None)
        per_cm.__exit__(None, None, None)

    nc.compile()
    return nc


def _bf16_split(a):
    import ml_dtypes

    a = np.ascontiguousarray(a, np.float32)
    hi = a.astype(ml_dtypes.bfloat16)
    lo = (a - hi.astype(np.float32)).astype(ml_dtypes.bfloat16)
    return np.ascontiguousarray(hi), np.ascontiguousarray(lo)


def _shard_inputs(inputs, memory, enc_h, enc_c, E, Wm, W_lstm, U_lstm, b_lstm, Wa, Wfc, bfc):
    import ml_dtypes

    inputs = np.ascontiguousarray(inputs)
    Eh, El = _bf16_split(E)
    Wmh, Wml = _bf16_split(Wm)
    W1h, W1l = _bf16_split(W_lstm[:D])
    W2h, W2l = _bf16_split(W_lstm[D:])
    blh, bll = _bf16_split(b_lstm.reshape(1, G))
    Wah, Wal = _bf16_split(Wa)
    # phase-2 attention Wa: h rows pre-halved (H=2h), single bf16
    Wa_att = np.ascontiguousarray(Wa, np.float32).copy()
    Wa_att[:U] *= 0.5
    shared = {
        "Eh": Eh, "El": El,
        "Wmh": Wmh, "Wml": Wml,
        "W1h": W1h, "W1l": W1l,
        "W2h": W2h, "W2l": W2l,
        "Ul": np.ascontiguousarray(U_lstm, np.float32),
        "blh": blh, "bll": bll,
        "Wah": Wah, "Wal": Wal,
        "Waatt": Wa_att.astype(ml_dtypes.bfloat16),
        "Wfc": np.ascontiguousarray(Wfc, np.float32).astype(ml_dtypes.bfloat16),
    }
    in_maps = []
    for rk in range(NCORES):
        sl = slice(BL * rk, BL * (rk + 1))
        m = dict(shared)
        m["tok_ids"] = np.ascontiguousarray(inputs[sl].T.reshape(NTOK, 1), np.int32)
        m["mem"] = np.ascontiguousarray(memory[sl], np.float32)
        hth, htl = _bf16_split(2.0 * enc_h[sl].T)
        m["enc_hth"] = hth
        m["enc_htl"] = htl
        m["enc_c2"] = np.ascontiguousarray(2.0 * enc_c[sl], np.float32)
        in_maps.append(m)
    return in_maps


def kernel(**inputs):
    from concourse.bass_utils import run_bass_kernel_spmd

    if "nc" not in _cache:
        _cache["nc"] = _build(T)
    nc = _cache["nc"]

    in_maps = _shard_inputs(**inputs)
    res = run_bass_kernel_spmd(nc, in_maps, core_ids=list(range(NCORES)))
    outs = []
    for rk in range(NCORES):
        o = np.asarray(res.results[rk]["out"], dtype=np.float32)  # [512, 32001]
        outs.append(o.reshape(T, BL, VO).transpose(1, 0, 2))
    full = np.concatenate(outs, axis=0)  # [32, 128, 32001]
    full += np.asarray(inputs["bfc"], np.float32)[None, None, :]
    return full
